# revision 39
# baseline (speedup 1.0000x reference)
"""Trainium2 Bass kernel for nn_MDRMWithCPRecon (optimized v2).

Sharding: pure data parallel over batch B=8 -> one batch element per
NeuronCore. All parameters replicated.

Per-core pipeline:
  x  = cat(frm, oth)                    [512, 64, 64]
  Fm = lrelu(conv3x3(x, W3) + b3)      [256, 64, 64]   <- bulk of FLOPs
  U1/U2/U3 rank-4 softmax factors from pooled stats
  spatial  = sigmoid(ws * U3 @ U2^T + bs)
  spectral = sigmoid(sigmoid(Wsa@mean + Wsm@max + biases))
  Wt = spectral x spatial
  fused    = a*Wt*frm + (1-a)*(1-Wt)*oth
  cp_recon = (Wr @ cp + br) * Wt + Fm,  cp = rank-4 CP(U1,U2,U3,lam)

v2 changes vs baseline (294us):
  - whole conv in bf16 (tolerance 2e-2; measured err stays ~1e-3):
    halves input DMA and SBUF, same 1 cycle/row PE rate as f32r.
  - inputs DMA'd straight into the padded SBUF image (bitcast view),
    no staging tiles / tensor_copy casts; borders memset once.
  - DMA order interleaves per-kt weight and image chunks so the PE can
    start accumulating ~2us in and is fed at matched rate.
  - Fm stays in SBUF as bf16 (kills the 8MB fm_scratch DRAM roundtrip).
  - E' = ((1-a)/a)*oth - frm precomputed on Vector during the conv.
  - pooled-stats -> U1/U2/U3 softmax chain batched into one [4, 384]
    tile (one exp, one ones-matmul row-sum, one reciprocal, one
    broadcast) instead of ~90 tiny serialized ops.
  - final elementwise stage in bf16 (2x DVE), spread over
    Vector/Scalar/GpSimd; outputs stored bf16, cast to f32 on host.
"""

import numpy as np
import ml_dtypes

import concourse.bacc as bacc
import concourse.bass as bass
import concourse.tile as tile
from concourse import mybir, bass_utils

F32 = mybir.dt.float32
BF16 = mybir.dt.bfloat16
AF = mybir.ActivationFunctionType
ALU = mybir.AluOpType
AX = mybir.AxisListType

B, C, H, W, K = 8, 256, 64, 64, 4
HW = H * W
NCORES = 8
BF = ml_dtypes.bfloat16

# tap order: center tap first and a full-window tap last (PSUM start/stop)
TAPS = [(1, 1), (0, 0), (0, 2), (2, 0), (2, 2), (0, 1), (1, 2), (1, 0),
        (2, 1)]


def build_program(alpha, ws, bs):
    from concourse.masks import make_identity

    nc = bacc.Bacc("TRN2", target_bir_lowering=False, debug=False,
                   num_devices=NCORES)

    # ---- DRAM I/O (per core) ----
    frm_d = nc.dram_tensor("frm", [128, 2, H, W], BF16, kind="ExternalInput")
    oth_d = nc.dram_tensor("oth", [128, 2, H, W], BF16, kind="ExternalInput")
    w3t_d = nc.dram_tensor("w3t", [128, 4, 2, 9, 128], BF16,
                           kind="ExternalInput")
    b3_d = nc.dram_tensor("b3", [128, 2], F32, kind="ExternalInput")
    wa_d = nc.dram_tensor("wa", [1, 2, 3, 256], F32, kind="ExternalInput")
    wu_d = nc.dram_tensor("wu", [128, 2, 4], F32, kind="ExternalInput")
    cu_d = nc.dram_tensor("cu", [4, 384], F32, kind="ExternalInput")
    wrt_d = nc.dram_tensor("wrt", [128, 2, 256], F32, kind="ExternalInput")
    br_d = nc.dram_tensor("br", [128, 2], F32, kind="ExternalInput")
    wsc_d = nc.dram_tensor("wsc", [128, 4, 2, 128], F32, kind="ExternalInput")
    bsc_d = nc.dram_tensor("bsc", [128, 2], F32, kind="ExternalInput")
    lam_d = nc.dram_tensor("lam", [4, 1], F32, kind="ExternalInput")
    fused_o = nc.dram_tensor("fused", [128, 2, H, W], BF16,
                             kind="ExternalOutput")
    cpr_o = nc.dram_tensor("cpr", [128, 2, H, W], BF16,
                           kind="ExternalOutput")

    with tile.TileContext(nc) as tc:
        _build_tile(tc, nc, make_identity, locals(), alpha, ws, bs)
    nc.compile()
    return nc


def _build_tile(tc, nc, make_identity, T, alpha, ws, bs):
    frm_d, oth_d, w3t_d, b3_d = T["frm_d"], T["oth_d"], T["w3t_d"], T["b3_d"]
    wa_d, wu_d, cu_d, wrt_d = T["wa_d"], T["wu_d"], T["cu_d"], T["wrt_d"]
    br_d, wsc_d, bsc_d, lam_d = T["br_d"], T["wsc_d"], T["bsc_d"], T["lam_d"]
    fused_o, cpr_o = T["fused_o"], T["cpr_o"]

    import contextlib
    ctx = contextlib.ExitStack()
    consts = ctx.enter_context(tc.tile_pool(name="consts", bufs=1))
    ew = ctx.enter_context(tc.tile_pool(name="ew", bufs=3))
    outr = ctx.enter_context(tc.tile_pool(name="outr", bufs=3))
    ps_conv = ctx.enter_context(tc.tile_pool(name="ps_conv", bufs=2,
                                             space="PSUM"))
    ps_sm = ctx.enter_context(tc.tile_pool(name="ps_sm", bufs=2,
                                           space="PSUM"))
    ps_fin = ctx.enter_context(tc.tile_pool(name="ps_fin", bufs=4,
                                            space="PSUM"))

    # ---- persistent SBUF tiles ----
    xin = consts.tile([128, 4, 66, 64], BF16)     # row-padded cat(frm,oth)
    w3t = consts.tile([128, 4, 2, 9, 128], BF16)  # conv weights
    fm = consts.tile([128, 2, 8, 512], BF16)      # conv output Fm
    e_sb = consts.tile([128, 2, 8, 512], BF16)    # E' = k*oth - frm
    t2_sb = consts.tile([128, 2, 8, 512], BF16)   # (1-a)*oth
    b3_sb = consts.tile([128, 2], F32)
    wa_sb = consts.tile([1, 2, 3, 256], F32)
    wu_sb = consts.tile([128, 2, 4], F32)
    cu_sb = consts.tile([4, 384], F32)
    wrt_sb = consts.tile([128, 2, 256], F32)
    br_sb = consts.tile([128, 2], F32)
    wsc_sb = consts.tile([128, 4, 2, 128], F32)
    bsc_sb = consts.tile([128, 2], F32)
    lam_sb = consts.tile([4, 1], F32)
    # stats
    sums1 = consts.tile([128, 2, 8], F32)      # per-(ct,pt) channel sums
    pp_sum = consts.tile([128, 2, 8, 64], F32)  # per-pt sum over h -> [c,w]
    pp_max = consts.tile([128, 2, 8, 64], F32)
    pq_sum = consts.tile([128, 2, 2, 64], F32)  # [m2|m3][ct] pooled sums
    pq_max = consts.tile([128, 2, 2, 64], F32)
    stat1 = consts.tile([128, 4], F32)          # [sum ct0, sum ct1, max...]
    # U chain (pooled rows kept on partition 0 only)
    row1s = consts.tile([1, 256], F32)
    row1m = consts.tile([1, 256], F32)
    row23s = consts.tile([1, 128], F32)
    row23m = consts.tile([1, 128], F32)
    a_sb = consts.tile([128, 2, 384], F32)
    ub_sb = consts.tile([4, 384], F32)
    ue_sb = consts.tile([4, 384], F32)
    rec_sb = consts.tile([1, 384], F32)
    u_all = consts.tile([4, 384], F32)
    u1n = consts.tile([128, 2, 4], F32)
    gag = consts.tile([128, 4], F32)
    spec = consts.tile([128, 2], F32)
    spcA = consts.tile([128, 2], F32)
    mx2 = consts.tile([128, 128], F32)
    G = consts.tile([4, HW], BF16)    # spatial: U3[h]*U2[w]
    Gc = consts.tile([4, HW], BF16)   # cp recon: U2[h]*U3[w]
    Gs = consts.tile([4, HW], BF16)   # Gc * s  (spatial sigmoid folded)
    s4 = consts.tile([4, HW], BF16)   # sigmoid spatial map, 4 rows
    MT = consts.tile([4, 256], BF16)
    brsr = consts.tile([1, 256], BF16)
    spcAb = consts.tile([1, 256], BF16)

    ident = consts.tile([128, 128], F32)
    make_identity(nc, ident[:])
    ones128 = consts.tile([128, 1], F32)
    nc.gpsimd.memset(ones128[:], 1.0)
    ones41 = consts.tile([4, 1], F32)
    nc.gpsimd.memset(ones41[:], 1.0)
    ones14 = consts.tile([1, 4], F32)
    nc.gpsimd.memset(ones14[:], 1.0)
    ones44b = consts.tile([4, 4], BF16)
    nc.gpsimd.memset(ones44b[:], 1.0)
    ones1x128b = consts.tile([1, 128], BF16)
    nc.gpsimd.memset(ones1x128b[:], 1.0)

    # ---- zero the padded top/bottom rows (cols handled per-tap) ----
    nc.gpsimd.memset(xin[:, :, 0:1, :], 0.0)
    nc.gpsimd.memset(xin[:, :, 65:66, :], 0.0)

    # ---- DMA order: b3 first, then per-kt (weights ct0, image h0) pairs,
    # then ct1 weights, h1 image halves, then smalls ----
    srcs = [frm_d, frm_d, oth_d, oth_d]
    for kt in range(4):
        nc.scalar.dma_start(w3t[:, kt, 0], w3t_d[:, kt, 0])
        nc.sync.dma_start(xin[:, kt, 1:33, :],
                          srcs[kt][:, kt % 2, 0:32, :])
        if kt == 0:
            nc.scalar.dma_start(b3_sb[:], b3_d[:])
    for kt in range(4):
        nc.scalar.dma_start(w3t[:, kt, 1], w3t_d[:, kt, 1])
    for kt in range(4):
        nc.sync.dma_start(xin[:, kt, 33:65, :],
                          srcs[kt][:, kt % 2, 32:64, :])
    for sb, dd in ((wa_sb, wa_d), (wu_sb, wu_d), (cu_sb, cu_d),
                   (wrt_sb, wrt_d), (br_sb, br_d), (wsc_sb, wsc_d),
                   (bsc_sb, bsc_d), (lam_sb, lam_d)):
        nc.scalar.dma_start(sb[:], dd[:])

    kk_e = float((1.0 - alpha) / alpha)

    # ---- conv3x3 + lrelu + streaming stats ----
    for pt in range(8):
        for ct in range(2):
            ps = ps_conv.tile([128, 8, 64], F32, tag="conv")
            idx = 0
            for kt in range(4):
                for (dy, dx) in TAPS:
                    # column edge handling: dx=0 drops out-col 0, dx=2
                    # drops out-col 63 (zero contribution at the border)
                    co0, co1 = (1, 64) if dx == 0 else (0, 63) \
                        if dx == 2 else (0, 64)
                    ci0 = dx - 1 + co0
                    nc.tensor.matmul(
                        ps[:, :, co0:co1],
                        w3t[:, kt, ct, dy * 3 + dx, :],
                        xin[:, kt, pt * 8 + dy: pt * 8 + dy + 8,
                            ci0: ci0 + co1 - co0],
                        start=(idx == 0), stop=(idx == 35))
                    idx += 1
            nc.scalar.activation(fm[:, ct, pt].rearrange(
                "p (h w) -> p h w", h=8), ps[:], AF.Lrelu,
                bias=b3_sb[:, ct:ct + 1], alpha=0.01,
                accum_out=sums1[:, ct, pt:pt + 1])
            blk = fm[:, ct, pt].rearrange("p (h w) -> p h w", h=8)
            blk_t = fm[:, ct, pt].rearrange("p (h w) -> p w h", h=8)
            # mode3 (per-h) stats: disjoint slices, written directly
            nc.vector.tensor_reduce(pq_sum[:, 1, ct, pt * 8:(pt + 1) * 8],
                                    blk, axis=AX.X, op=ALU.add)
            nc.vector.tensor_reduce(pq_max[:, 1, ct, pt * 8:(pt + 1) * 8],
                                    blk, axis=AX.X, op=ALU.max)
            # mode2 (per-w) partials, combined after the loop
            nc.vector.tensor_reduce(pp_sum[:, ct, pt, :], blk_t,
                                    axis=AX.X, op=ALU.add)
            nc.vector.tensor_reduce(pp_max[:, ct, pt, :], blk_t,
                                    axis=AX.X, op=ALU.max)
            # E'/t2 precompute (no conv dependency; fills idle slots)
            nc.vector.scalar_tensor_tensor(
                e_sb[:, ct, pt].rearrange("p (h w) -> p h w", h=8),
                xin[:, 2 + ct, 1 + pt * 8: 9 + pt * 8, :], kk_e,
                xin[:, ct, 1 + pt * 8: 9 + pt * 8, :],
                op0=ALU.mult, op1=ALU.subtract)
            nc.vector.tensor_scalar(
                t2_sb[:, ct, pt].rearrange("p (h w) -> p h w", h=8),
                xin[:, 2 + ct, 1 + pt * 8: 9 + pt * 8, :],
                float(1.0 - alpha), None, op0=ALU.mult)

    # ---- combine stats (global max = max over h of per-h maxes) ----
    nc.vector.tensor_reduce(stat1[:, 0:2], sums1[:], axis=AX.X, op=ALU.add)
    nc.vector.tensor_reduce(stat1[:, 2:4], pq_max[:, 1], axis=AX.X,
                            op=ALU.max)
    nc.vector.tensor_reduce(
        pq_sum[:, 0], pp_sum[:].rearrange("p c t w -> p c w t"),
        axis=AX.X, op=ALU.add)
    nc.vector.tensor_reduce(
        pq_max[:, 0], pp_max[:].rearrange("p c t w -> p c w t"),
        axis=AX.X, op=ALU.max)

    # mode1 rows via per-column transposes; avg-scales folded into wa
    rowdst = [(row1s, 0), (row1s, 128), (row1m, 0), (row1m, 128)]
    for j, (dst, off) in enumerate(rowdst):
        tpj = ps_sm.tile([1, 128], F32, tag="sm")
        nc.tensor.transpose(tpj[:], stat1[:, j:j + 1], ident[:])
        nc.scalar.copy(dst[0:1, off:off + 128], tpj[:])

    # mode2/3 sum rows: ones-matmul over channels, add ct halves
    srow = ps_sm.tile([1, 2, 2, 64], F32, tag="sm")
    nc.tensor.matmul(srow[:].rearrange("p a b c -> p (a b c)"), ones128[:],
                     pq_sum[:].rearrange("p a b c -> p (a b c)"),
                     start=True, stop=True)
    sr_sb = consts.tile([1, 2, 2, 64], F32)
    nc.scalar.copy(sr_sb[:], srow[:])
    nc.vector.tensor_tensor(row23s[0:1, :].rearrange("p (m w) -> p m w",
                                                     m=2),
                            sr_sb[:, :, 0, :], sr_sb[:, :, 1, :], op=ALU.add)
    # mode2/3 max rows: ct-combine, transpose, reduce, transpose back
    nc.vector.tensor_tensor(mx2[:].rearrange("p (m w) -> p m w", m=2),
                            pq_max[:, :, 0, :], pq_max[:, :, 1, :],
                            op=ALU.max)
    mxT = ps_sm.tile([128, 128], F32, tag="sm")
    nc.tensor.transpose(mxT[:], mx2[:], ident[:])
    mcol = ew.tile([128, 1], F32, tag="mcol")
    nc.vector.tensor_reduce(mcol[:], mxT[:], axis=AX.X, op=ALU.max)
    mrow = ps_sm.tile([1, 128], F32, tag="sm")
    nc.tensor.transpose(mrow[:], mcol[:], ident[:])
    nc.scalar.copy(row23m[:], mrow[:])

    # ---- a[o, n] = sum_s wa_s[o] * row_s[n]  (rank-1 outer products) ----
    for ct in range(2):
        ap_t = ps_sm.tile([128, 384], F32, tag="sm")
        cs = ct * 128
        nc.tensor.matmul(ap_t[:, 0:256], wa_sb[0:1, 0, 0, cs:cs + 128],
                         row1s[:], start=True, stop=False)
        nc.tensor.matmul(ap_t[:, 0:256], wa_sb[0:1, 1, 0, cs:cs + 128],
                         row1m[:], start=False, stop=True)
        nc.tensor.matmul(ap_t[:, 256:320], wa_sb[0:1, 0, 1, cs:cs + 128],
                         row23s[:, 0:64], start=True, stop=False)
        nc.tensor.matmul(ap_t[:, 256:320], wa_sb[0:1, 1, 1, cs:cs + 128],
                         row23m[:, 0:64], start=False, stop=True)
        nc.tensor.matmul(ap_t[:, 320:384], wa_sb[0:1, 0, 2, cs:cs + 128],
                         row23s[:, 64:128], start=True, stop=False)
        nc.tensor.matmul(ap_t[:, 320:384], wa_sb[0:1, 1, 2, cs:cs + 128],
                         row23m[:, 64:128], start=False, stop=True)
        nc.scalar.copy(a_sb[:, ct, :], ap_t[:])

    # ---- u = Wu @ a + (Wu@ba + bu)  [4, 384], then softmax over k ----
    u_ps = ps_sm.tile([4, 384], F32, tag="sm")
    nc.tensor.matmul(u_ps[:], wu_sb[:, 0, :], a_sb[:, 0, :], start=True,
                     stop=False)
    nc.tensor.matmul(u_ps[:], wu_sb[:, 1, :], a_sb[:, 1, :], start=False,
                     stop=True)
    nc.vector.tensor_tensor(ub_sb[:], u_ps[:], cu_sb[:], op=ALU.add)
    nc.scalar.activation(ue_sb[:], ub_sb[:], AF.Exp)
    ssum = ps_sm.tile([1, 384], F32, tag="sm")
    nc.tensor.matmul(ssum[:], ones41[:], ue_sb[:], start=True, stop=True)
    nc.scalar.copy(rec_sb[:], ssum[:])
    rb = ps_sm.tile([4, 384], F32, tag="sm")
    nc.tensor.matmul(rb[:], ones14[:], rec_sb[:], start=True, stop=True)
    rcp4 = consts.tile([4, 384], F32)
    nc.vector.reciprocal_approx_fast(rcp4[:], rb[:])
    nc.vector.tensor_tensor(u_all[:], ue_sb[:], rcp4[:], op=ALU.mult)

    # ---- G[r, h, w] = U3[h, r] * U2[w, r]  (bf16, spatial) ----
    nc.vector.tensor_tensor(
        G[:].rearrange("p (h w) -> p h w", h=64),
        u_all[:, 320:384][:, :, None].broadcast_to([4, 64, 64]),
        u_all[:, 256:320][:, None, :].broadcast_to([4, 64, 64]),
        op=ALU.mult)
    # ---- Gc[r, h, w] = U2[h, r] * U3[w, r]  (bf16, cp recon) ----
    nc.vector.tensor_tensor(
        Gc[:].rearrange("p (h w) -> p h w", h=64),
        u_all[:, 256:320][:, :, None].broadcast_to([4, 64, 64]),
        u_all[:, 320:384][:, None, :].broadcast_to([4, 64, 64]),
        op=ALU.mult)

    # ---- MT = (Wr @ U1 diag(lam))^T  [4, 256] bf16 ----
    for kk2 in range(2):
        u1t_ps = ps_sm.tile([128, 4], F32, tag="sm")
        nc.tensor.transpose(u1t_ps[:], u_all[0:4, kk2 * 128:(kk2 + 1) * 128],
                            ident[0:4, 0:4])
        nc.scalar.copy(u1n[:, kk2, :], u1t_ps[:])
    mt_ps = ps_sm.tile([4, 256], F32, tag="sm")
    nc.tensor.matmul(mt_ps[:], u1n[:, 0, :], wrt_sb[:, 0, :], start=True,
                     stop=False)
    nc.tensor.matmul(mt_ps[:], u1n[:, 1, :], wrt_sb[:, 1, :], start=False,
                     stop=True)
    nc.vector.tensor_scalar(MT[:], mt_ps[:], lam_sb[:], None, op0=ALU.mult)

    # ---- spectral attention ----
    f_ps = ps_sm.tile([128, 2, 128], F32, tag="sm")
    for ct in range(2):
        nc.tensor.matmul(f_ps[:, ct, :],
                         u_all[0:4, ct * 128:(ct + 1) * 128],
                         u_all[:, 256:384], start=True, stop=True)
        nc.vector.tensor_reduce(gag[:, ct:ct + 1], f_ps[:, ct, :],
                                axis=AX.X, op=ALU.add)
        nc.vector.tensor_reduce(gag[:, 2 + ct:3 + ct], f_ps[:, ct, :],
                                axis=AX.X, op=ALU.max)
    spv = ps_sm.tile([128, 2], F32, tag="sm")
    for mm in range(2):
        for kk2 in range(4):
            nc.tensor.matmul(spv[:, mm:mm + 1], wsc_sb[:, kk2, mm, :],
                             gag[:, kk2:kk2 + 1], start=(kk2 == 0),
                             stop=(kk2 == 3))
    for mm in range(2):
        stmp = ew.tile([128, 1], F32, tag="stmp")
        nc.scalar.activation(stmp[:], spv[:, mm:mm + 1], AF.Sigmoid,
                             bias=bsc_sb[:, mm:mm + 1])
        nc.scalar.activation(spec[:, mm:mm + 1], stmp[:], AF.Sigmoid)
    nc.vector.tensor_scalar(spcA[:], spec[:], float(-alpha), None,
                            op0=ALU.mult)
    # brs row: (br * spectral) as a [1, 256] bf16 row for the rank-1 term
    brs = ew.tile([128, 2], F32, tag="brs")
    nc.vector.tensor_tensor(brs[:], br_sb[:], spec[:], op=ALU.mult)
    for ct in range(2):
        brt = ps_sm.tile([1, 128], F32, tag="sm")
        nc.tensor.transpose(brt[:], brs[:, ct:ct + 1], ident[:])
        nc.vector.tensor_copy(brsr[0:1, ct * 128:(ct + 1) * 128], brt[:])
        sat = ps_sm.tile([1, 128], F32, tag="sm")
        nc.tensor.transpose(sat[:], spcA[:, ct:ct + 1], ident[:])
        nc.vector.tensor_copy(spcAb[0:1, ct * 128:(ct + 1) * 128], sat[:])

    # ---- final stage ----
    # fused  = t2 + spcA*(E' . s);  cp = spec*((MT@Gc).s + br.s) + Fm
    for pt in range(8):
        sl = slice(pt * 512, (pt + 1) * 512)
        srow_ps = ps_fin.tile([4, 512], F32, tag="sb2")
        nc.tensor.matmul(srow_ps[:], ones44b[:], G[:, sl], start=True,
                         stop=True)
        nc.scalar.activation(s4[:, sl], srow_ps[:], AF.Sigmoid,
                             scale=float(ws), bias=float(bs))
        nc.vector.tensor_tensor(Gs[:, sl], Gc[:, sl], s4[:, sl],
                                op=ALU.mult)
        wts = []
        for ct in range(2):
            wt_ps = ps_fin.tile([128, 512], F32, tag="sb2")
            nc.tensor.matmul(wt_ps[:],
                             spcAb[0:1, ct * 128:(ct + 1) * 128],
                             s4[0:1, sl], start=True, stop=True)
            wts.append(wt_ps)
        rcs = []
        for ct in range(2):
            rc_ps = ps_conv.tile([128, 512], F32, tag="conv")
            nc.tensor.matmul(rc_ps[:], MT[:, ct * 128:(ct + 1) * 128],
                             Gs[:, sl], start=True, stop=False)
            nc.tensor.matmul(rc_ps[:], brsr[0:1, ct * 128:(ct + 1) * 128],
                             s4[0:1, sl], start=False, stop=True)
            rcs.append(rc_ps)
        for ct in range(2):
            fa = ew.tile([128, 512], BF16, tag="fa")
            nc.vector.tensor_tensor(fa[:], e_sb[:, ct, pt, :], wts[ct][:],
                                    op=ALU.mult)
            fu = outr.tile([128, 8, 64], BF16, tag="fu")
            nc.vector.tensor_tensor(
                fu[:], fa[:].rearrange("p (h w) -> p h w", h=8),
                t2_sb[:, ct, pt].rearrange("p (h w) -> p h w", h=8),
                op=ALU.add)
            nc.sync.dma_start(fused_o[:, ct, pt * 8:(pt + 1) * 8, :], fu[:])
        for ct in range(2):
            cp = outr.tile([128, 8, 64], BF16, tag="cp")
            nc.vector.scalar_tensor_tensor(
                cp[:], rcs[ct][:].rearrange("p (h w) -> p h w", h=8),
                spec[:, ct:ct + 1],
                fm[:, ct, pt].rearrange("p (h w) -> p h w", h=8),
                op0=ALU.mult, op1=ALU.add)
            nc.sync.dma_start(cpr_o[:, ct, pt * 8:(pt + 1) * 8, :], cp[:])
    ctx.close()


def _prep_weights(W3, b3, Wa1, ba1, Wa2, ba2, Wa3, ba3, Wu, bu, Wr, br,
                  Wsa, bsa, Wsm, bsm):
    f = np.float32
    # w3t[p, kt, ct, t, co] = W3[ct*128+co, kt*128+p, dy, dx]
    w3t = np.ascontiguousarray(
        W3.reshape(2, 128, 4, 128, 9).transpose(3, 2, 0, 4, 1)).astype(BF)
    b3h = np.ascontiguousarray(b3.reshape(2, 128).T).astype(f)
    # wa[0, s, m, o] = Wa_m[o, s]; avg column scaled by 1/pool_n
    was = []
    for m, wv in enumerate((Wa1, Wa2, Wa3)):
        wv = np.array(wv, f).copy()
        wv[:, 0] /= (HW if m == 0 else C * H)
        was.append(wv)
    wa = np.ascontiguousarray(
        np.stack(was, axis=0).transpose(2, 0, 1)[None]).astype(f)
    # wu[p, ct, k] = Wu[k, ct*128+p]
    wu = np.ascontiguousarray(
        Wu.reshape(K, 2, 128).transpose(2, 1, 0)).astype(f)
    # cu[k, n] = (Wu @ ba_m + bu)[k] for n in mode-m block
    cus = [Wu @ bam + bu for bam in (ba1, ba2, ba3)]
    cu = np.concatenate([np.tile(cus[0][:, None], (1, 256)),
                         np.tile(cus[1][:, None], (1, 64)),
                         np.tile(cus[2][:, None], (1, 64))], axis=1)
    cu = np.ascontiguousarray(cu).astype(f)
    # wrt[p, kk, m] = Wr[m, kk*128+p]
    wrt = np.ascontiguousarray(
        Wr.reshape(256, 2, 128).transpose(2, 1, 0)).astype(f)
    brh = np.ascontiguousarray(br.reshape(2, 128).T).astype(f)
    # wsc[p, kk, mm, m]: kk<2 -> Wsa/(W+H) (mean folded), kk>=2 -> Wsm
    wsa_r = (Wsa / 128.0).reshape(2, 128, 2, 128).transpose(3, 2, 0, 1)
    wsm_r = Wsm.reshape(2, 128, 2, 128).transpose(3, 2, 0, 1)
    wsc = np.ascontiguousarray(
        np.concatenate([wsa_r, wsm_r], axis=1)).astype(f)
    bsc = np.ascontiguousarray((bsa + bsm).reshape(2, 128).T).astype(f)
    return dict(w3t=w3t, b3=b3h, wa=wa, wu=wu, cu=cu, wrt=wrt, br=brh,
                wsc=wsc, bsc=bsc)


_CACHE = {}


def kernel(frm_feat, other_feat, W3, b3, Wa1, ba1, Wa2, ba2, Wa3, ba3,
           Wu, bu, Wr, br, ws, bs, Wsa, bsa, Wsm, bsm, alpha, lam,
           _trace=False, _tmpdir=None):
    key = (float(alpha), float(ws), float(bs))
    if key not in _CACHE:
        _CACHE[key] = build_program(float(alpha), float(ws), float(bs))
    nc = _CACHE[key]

    wd = _prep_weights(np.asarray(W3, np.float32), np.asarray(b3),
                       np.asarray(Wa1), np.asarray(ba1), np.asarray(Wa2),
                       np.asarray(ba2), np.asarray(Wa3), np.asarray(ba3),
                       np.asarray(Wu), np.asarray(bu),
                       np.asarray(Wr, np.float32), np.asarray(br),
                       np.asarray(Wsa, np.float32), np.asarray(bsa),
                       np.asarray(Wsm, np.float32), np.asarray(bsm))
    wd["lam"] = np.asarray(lam, np.float32).reshape(4, 1)

    in_maps = []
    for b_i in range(NCORES):
        m = dict(wd)
        m["frm"] = frm_bat(frm_feat, b_i)
        m["oth"] = frm_bat(other_feat, b_i)
        in_maps.append(m)

    res = bass_utils.run_bass_kernel_spmd(
        nc, in_maps, core_ids=list(range(NCORES)), trace=_trace,
        tmpdir=_tmpdir)
    fused = np.stack([_unshard(res.results[i]["fused"])
                      for i in range(NCORES)])
    cpr = np.stack([_unshard(res.results[i]["cpr"])
                    for i in range(NCORES)])
    kernel._last_exec_time_ns = res.exec_time_ns
    kernel._last_results = res
    return fused, cpr


def frm_bat(x, b_i):
    """[B, 256, H, W] f32 -> [128, 2, H, W] bf16 for batch b_i."""
    return np.ascontiguousarray(
        np.asarray(x[b_i], np.float32).reshape(2, 128, H, W)
        .transpose(1, 0, 2, 3)).astype(BF)


def _unshard(a):
    """[128, 2, H, W] bf16 -> [256, H, W] f32."""
    return np.ascontiguousarray(
        np.asarray(a).transpose(1, 0, 2, 3)).reshape(256, H, W)\
        .astype(np.float32)


# revision 41
# speedup vs baseline: 1.0048x; 1.0048x over previous
"""Trainium2 Bass kernel for nn_MDRMWithCPRecon (optimized v2).

Sharding: pure data parallel over batch B=8 -> one batch element per
NeuronCore. All parameters replicated.

Per-core pipeline:
  x  = cat(frm, oth)                    [512, 64, 64]
  Fm = lrelu(conv3x3(x, W3) + b3)      [256, 64, 64]   <- bulk of FLOPs
  U1/U2/U3 rank-4 softmax factors from pooled stats
  spatial  = sigmoid(ws * U3 @ U2^T + bs)
  spectral = sigmoid(sigmoid(Wsa@mean + Wsm@max + biases))
  Wt = spectral x spatial
  fused    = a*Wt*frm + (1-a)*(1-Wt)*oth
  cp_recon = (Wr @ cp + br) * Wt + Fm,  cp = rank-4 CP(U1,U2,U3,lam)

v2 changes vs baseline (294us):
  - whole conv in bf16 (tolerance 2e-2; measured err stays ~1e-3):
    halves input DMA and SBUF, same 1 cycle/row PE rate as f32r.
  - inputs DMA'd straight into the padded SBUF image (bitcast view),
    no staging tiles / tensor_copy casts; borders memset once.
  - DMA order interleaves per-kt weight and image chunks so the PE can
    start accumulating ~2us in and is fed at matched rate.
  - Fm stays in SBUF as bf16 (kills the 8MB fm_scratch DRAM roundtrip).
  - E' = ((1-a)/a)*oth - frm precomputed on Vector during the conv.
  - pooled-stats -> U1/U2/U3 softmax chain batched into one [4, 384]
    tile (one exp, one ones-matmul row-sum, one reciprocal, one
    broadcast) instead of ~90 tiny serialized ops.
  - final elementwise stage in bf16 (2x DVE), spread over
    Vector/Scalar/GpSimd; outputs stored bf16, cast to f32 on host.
"""

import numpy as np
import ml_dtypes

import concourse.bacc as bacc
import concourse.bass as bass
import concourse.tile as tile
from concourse import mybir, bass_utils

F32 = mybir.dt.float32
BF16 = mybir.dt.bfloat16
AF = mybir.ActivationFunctionType
ALU = mybir.AluOpType
AX = mybir.AxisListType

B, C, H, W, K = 8, 256, 64, 64, 4
HW = H * W
NCORES = 8
BF = ml_dtypes.bfloat16

# tap order: center tap first and a full-window tap last (PSUM start/stop)
TAPS = [(1, 1), (0, 0), (0, 2), (2, 0), (2, 2), (0, 1), (1, 2), (1, 0),
        (2, 1)]


def build_program(alpha, ws, bs):
    from concourse.masks import make_identity

    nc = bacc.Bacc("TRN2", target_bir_lowering=False, debug=False,
                   num_devices=NCORES)

    # ---- DRAM I/O (per core) ----
    frm_d = nc.dram_tensor("frm", [128, 2, H, W], BF16, kind="ExternalInput")
    oth_d = nc.dram_tensor("oth", [128, 2, H, W], BF16, kind="ExternalInput")
    w3t_d = nc.dram_tensor("w3t", [128, 4, 2, 9, 128], BF16,
                           kind="ExternalInput")
    b3_d = nc.dram_tensor("b3", [128, 2], F32, kind="ExternalInput")
    wa_d = nc.dram_tensor("wa", [1, 2, 3, 256], F32, kind="ExternalInput")
    wu_d = nc.dram_tensor("wu", [128, 2, 4], F32, kind="ExternalInput")
    cu_d = nc.dram_tensor("cu", [4, 384], F32, kind="ExternalInput")
    wrt_d = nc.dram_tensor("wrt", [128, 2, 256], F32, kind="ExternalInput")
    br_d = nc.dram_tensor("br", [128, 2], F32, kind="ExternalInput")
    wsc_d = nc.dram_tensor("wsc", [128, 4, 2, 128], F32, kind="ExternalInput")
    bsc_d = nc.dram_tensor("bsc", [128, 2], F32, kind="ExternalInput")
    lam_d = nc.dram_tensor("lam", [4, 1], F32, kind="ExternalInput")
    fused_o = nc.dram_tensor("fused", [128, 2, H, W], BF16,
                             kind="ExternalOutput")
    cpr_o = nc.dram_tensor("cpr", [128, 2, H, W], BF16,
                           kind="ExternalOutput")

    with tile.TileContext(nc) as tc:
        _build_tile(tc, nc, make_identity, locals(), alpha, ws, bs)
    nc.compile()
    return nc


def _build_tile(tc, nc, make_identity, T, alpha, ws, bs):
    frm_d, oth_d, w3t_d, b3_d = T["frm_d"], T["oth_d"], T["w3t_d"], T["b3_d"]
    wa_d, wu_d, cu_d, wrt_d = T["wa_d"], T["wu_d"], T["cu_d"], T["wrt_d"]
    br_d, wsc_d, bsc_d, lam_d = T["br_d"], T["wsc_d"], T["bsc_d"], T["lam_d"]
    fused_o, cpr_o = T["fused_o"], T["cpr_o"]

    import contextlib
    ctx = contextlib.ExitStack()
    consts = ctx.enter_context(tc.tile_pool(name="consts", bufs=1))
    ew = ctx.enter_context(tc.tile_pool(name="ew", bufs=3))
    outr = ctx.enter_context(tc.tile_pool(name="outr", bufs=3))
    ps_conv = ctx.enter_context(tc.tile_pool(name="ps_conv", bufs=2,
                                             space="PSUM"))
    ps_sm = ctx.enter_context(tc.tile_pool(name="ps_sm", bufs=2,
                                           space="PSUM"))
    ps_fin = ctx.enter_context(tc.tile_pool(name="ps_fin", bufs=4,
                                            space="PSUM"))

    # ---- persistent SBUF tiles ----
    xin = consts.tile([128, 4, 66, 64], BF16)     # row-padded cat(frm,oth)
    w3t = consts.tile([128, 4, 2, 9, 128], BF16)  # conv weights
    fm = consts.tile([128, 2, 8, 512], BF16)      # conv output Fm
    e_sb = consts.tile([128, 2, 8, 512], BF16)    # E' = k*oth - frm
    t2_sb = consts.tile([128, 2, 8, 512], BF16)   # (1-a)*oth
    b3_sb = consts.tile([128, 2], F32)
    wa_sb = consts.tile([1, 2, 3, 256], F32)
    wu_sb = consts.tile([128, 2, 4], F32)
    cu_sb = consts.tile([4, 384], F32)
    wrt_sb = consts.tile([128, 2, 256], F32)
    br_sb = consts.tile([128, 2], F32)
    wsc_sb = consts.tile([128, 4, 2, 128], F32)
    bsc_sb = consts.tile([128, 2], F32)
    lam_sb = consts.tile([4, 1], F32)
    # stats
    sums1 = consts.tile([128, 2, 8], F32)      # per-(ct,pt) channel sums
    pp_sum = consts.tile([128, 2, 8, 64], F32)  # per-pt sum over h -> [c,w]
    pp_max = consts.tile([128, 2, 8, 64], F32)
    pq_sum = consts.tile([128, 2, 2, 64], F32)  # [m2|m3][ct] pooled sums
    pq_max = consts.tile([128, 2, 2, 64], F32)
    stat1 = consts.tile([128, 4], F32)          # [sum ct0, sum ct1, max...]
    # U chain (pooled rows kept on partition 0 only)
    row1s = consts.tile([1, 256], F32)
    row1m = consts.tile([1, 256], F32)
    row23s = consts.tile([1, 128], F32)
    row23m = consts.tile([1, 128], F32)
    a_sb = consts.tile([128, 2, 384], F32)
    ub_sb = consts.tile([4, 384], F32)
    ue_sb = consts.tile([4, 384], F32)
    rec_sb = consts.tile([1, 384], F32)
    u_all = consts.tile([4, 384], F32)
    u1n = consts.tile([128, 2, 4], F32)
    gag = consts.tile([128, 4], F32)
    spec = consts.tile([128, 2], F32)
    spcA = consts.tile([128, 2], F32)
    mx2 = consts.tile([128, 128], F32)
    G = consts.tile([4, HW], BF16)    # spatial: U3[h]*U2[w]
    Gc = consts.tile([4, HW], BF16)   # cp recon: U2[h]*U3[w]
    Gs = consts.tile([4, HW], BF16)   # Gc * s  (spatial sigmoid folded)
    s4 = consts.tile([4, HW], BF16)   # sigmoid spatial map, 4 rows
    MT = consts.tile([4, 256], BF16)
    brsr = consts.tile([1, 256], BF16)
    spcAb = consts.tile([1, 256], BF16)

    ident = consts.tile([128, 128], F32)
    make_identity(nc, ident[:])
    ones128 = consts.tile([128, 1], F32)
    nc.gpsimd.memset(ones128[:], 1.0)
    ones41 = consts.tile([4, 1], F32)
    nc.gpsimd.memset(ones41[:], 1.0)
    ones14 = consts.tile([1, 4], F32)
    nc.gpsimd.memset(ones14[:], 1.0)
    ones44b = consts.tile([4, 4], BF16)
    nc.gpsimd.memset(ones44b[:], 1.0)
    ones1x128b = consts.tile([1, 128], BF16)
    nc.gpsimd.memset(ones1x128b[:], 1.0)

    # ---- zero the padded top/bottom rows (cols handled per-tap) ----
    nc.gpsimd.memset(xin[:, :, 0:1, :], 0.0)
    nc.gpsimd.memset(xin[:, :, 65:66, :], 0.0)

    # ---- DMA order: b3 first, then per-kt (weights ct0, image h0) pairs,
    # then ct1 weights, h1 image halves, then smalls ----
    srcs = [frm_d, frm_d, oth_d, oth_d]
    for kt in range(4):
        nc.scalar.dma_start(w3t[:, kt, 0], w3t_d[:, kt, 0])
        nc.sync.dma_start(xin[:, kt, 1:33, :],
                          srcs[kt][:, kt % 2, 0:32, :])
        if kt == 0:
            nc.scalar.dma_start(b3_sb[:], b3_d[:])
    for kt in range(4):
        nc.scalar.dma_start(w3t[:, kt, 1], w3t_d[:, kt, 1])
    for kt in range(4):
        nc.sync.dma_start(xin[:, kt, 33:65, :],
                          srcs[kt][:, kt % 2, 32:64, :])
    for sb, dd in ((wa_sb, wa_d), (wu_sb, wu_d), (cu_sb, cu_d),
                   (wrt_sb, wrt_d), (br_sb, br_d), (wsc_sb, wsc_d),
                   (bsc_sb, bsc_d), (lam_sb, lam_d)):
        nc.scalar.dma_start(sb[:], dd[:])

    kk_e = float((1.0 - alpha) / alpha)

    # ---- conv3x3 + lrelu + streaming stats ----
    for pt in range(8):
        for ct in range(2):
            ps = ps_conv.tile([128, 8, 64], F32, tag="conv")
            idx = 0
            for kt in range(4):
                for (dy, dx) in TAPS:
                    # column edge handling: dx=0 drops out-col 0, dx=2
                    # drops out-col 63 (zero contribution at the border)
                    co0, co1 = (1, 64) if dx == 0 else (0, 63) \
                        if dx == 2 else (0, 64)
                    ci0 = dx - 1 + co0
                    nc.tensor.matmul(
                        ps[:, :, co0:co1],
                        w3t[:, kt, ct, dy * 3 + dx, :],
                        xin[:, kt, pt * 8 + dy: pt * 8 + dy + 8,
                            ci0: ci0 + co1 - co0],
                        start=(idx == 0), stop=(idx == 35))
                    idx += 1
            nc.scalar.activation(fm[:, ct, pt].rearrange(
                "p (h w) -> p h w", h=8), ps[:], AF.Lrelu,
                bias=b3_sb[:, ct:ct + 1], alpha=0.01,
                accum_out=sums1[:, ct, pt:pt + 1])
            blk = fm[:, ct, pt].rearrange("p (h w) -> p h w", h=8)
            blk_t = fm[:, ct, pt].rearrange("p (h w) -> p w h", h=8)
            # mode3 (per-h) stats: disjoint slices, written directly
            nc.vector.tensor_reduce(pq_sum[:, 1, ct, pt * 8:(pt + 1) * 8],
                                    blk, axis=AX.X, op=ALU.add)
            nc.vector.tensor_reduce(pq_max[:, 1, ct, pt * 8:(pt + 1) * 8],
                                    blk, axis=AX.X, op=ALU.max)
            # mode2 (per-w) partials, combined after the loop
            nc.vector.tensor_reduce(pp_sum[:, ct, pt, :], blk_t,
                                    axis=AX.X, op=ALU.add)
            nc.vector.tensor_reduce(pp_max[:, ct, pt, :], blk_t,
                                    axis=AX.X, op=ALU.max)
            # E'/t2 precompute (no conv dependency; fills idle slots)
            nc.vector.scalar_tensor_tensor(
                e_sb[:, ct, pt].rearrange("p (h w) -> p h w", h=8),
                xin[:, 2 + ct, 1 + pt * 8: 9 + pt * 8, :], kk_e,
                xin[:, ct, 1 + pt * 8: 9 + pt * 8, :],
                op0=ALU.mult, op1=ALU.subtract)
            nc.vector.tensor_scalar(
                t2_sb[:, ct, pt].rearrange("p (h w) -> p h w", h=8),
                xin[:, 2 + ct, 1 + pt * 8: 9 + pt * 8, :],
                float(1.0 - alpha), None, op0=ALU.mult)

    # ---- combine stats (global max = max over h of per-h maxes) ----
    nc.vector.tensor_reduce(stat1[:, 0:2], sums1[:], axis=AX.X, op=ALU.add)
    nc.vector.tensor_reduce(stat1[:, 2:4], pq_max[:, 1], axis=AX.X,
                            op=ALU.max)
    nc.vector.tensor_reduce(
        pq_sum[:, 0], pp_sum[:].rearrange("p c t w -> p c w t"),
        axis=AX.X, op=ALU.add)
    nc.vector.tensor_reduce(
        pq_max[:, 0], pp_max[:].rearrange("p c t w -> p c w t"),
        axis=AX.X, op=ALU.max)

    # mode1 rows via per-column transposes; avg-scales folded into wa
    rowdst = [(row1s, 0), (row1s, 128), (row1m, 0), (row1m, 128)]
    for j, (dst, off) in enumerate(rowdst):
        tpj = ps_sm.tile([1, 128], F32, tag="sm")
        nc.tensor.transpose(tpj[:], stat1[:, j:j + 1], ident[:])
        nc.scalar.copy(dst[0:1, off:off + 128], tpj[:])

    # mode2/3 sum rows: ones-matmul over channels, add ct halves
    srow = ps_sm.tile([1, 2, 2, 64], F32, tag="sm")
    nc.tensor.matmul(srow[:].rearrange("p a b c -> p (a b c)"), ones128[:],
                     pq_sum[:].rearrange("p a b c -> p (a b c)"),
                     start=True, stop=True)
    sr_sb = consts.tile([1, 2, 2, 64], F32)
    nc.scalar.copy(sr_sb[:], srow[:])
    nc.vector.tensor_tensor(row23s[0:1, :].rearrange("p (m w) -> p m w",
                                                     m=2),
                            sr_sb[:, :, 0, :], sr_sb[:, :, 1, :], op=ALU.add)
    # mode2/3 max rows: ct-combine, transpose, reduce, transpose back
    nc.vector.tensor_tensor(mx2[:].rearrange("p (m w) -> p m w", m=2),
                            pq_max[:, :, 0, :], pq_max[:, :, 1, :],
                            op=ALU.max)
    mxT = ps_sm.tile([128, 128], F32, tag="sm")
    nc.tensor.transpose(mxT[:], mx2[:], ident[:])
    mcol = ew.tile([128, 1], F32, tag="mcol")
    nc.vector.tensor_reduce(mcol[:], mxT[:], axis=AX.X, op=ALU.max)
    mrow = ps_sm.tile([1, 128], F32, tag="sm")
    nc.tensor.transpose(mrow[:], mcol[:], ident[:])
    nc.scalar.copy(row23m[:], mrow[:])

    # ---- a[o, n] = sum_s wa_s[o] * row_s[n]  (rank-1 outer products) ----
    for ct in range(2):
        ap_t = ps_sm.tile([128, 384], F32, tag="sm")
        cs = ct * 128
        nc.tensor.matmul(ap_t[:, 0:256], wa_sb[0:1, 0, 0, cs:cs + 128],
                         row1s[:], start=True, stop=False)
        nc.tensor.matmul(ap_t[:, 0:256], wa_sb[0:1, 1, 0, cs:cs + 128],
                         row1m[:], start=False, stop=True)
        nc.tensor.matmul(ap_t[:, 256:320], wa_sb[0:1, 0, 1, cs:cs + 128],
                         row23s[:, 0:64], start=True, stop=False)
        nc.tensor.matmul(ap_t[:, 256:320], wa_sb[0:1, 1, 1, cs:cs + 128],
                         row23m[:, 0:64], start=False, stop=True)
        nc.tensor.matmul(ap_t[:, 320:384], wa_sb[0:1, 0, 2, cs:cs + 128],
                         row23s[:, 64:128], start=True, stop=False)
        nc.tensor.matmul(ap_t[:, 320:384], wa_sb[0:1, 1, 2, cs:cs + 128],
                         row23m[:, 64:128], start=False, stop=True)
        nc.scalar.copy(a_sb[:, ct, :], ap_t[:])

    # ---- u = Wu @ a + (Wu@ba + bu)  [4, 384], then softmax over k ----
    u_ps = ps_sm.tile([4, 384], F32, tag="sm")
    nc.tensor.matmul(u_ps[:], wu_sb[:, 0, :], a_sb[:, 0, :], start=True,
                     stop=False)
    nc.tensor.matmul(u_ps[:], wu_sb[:, 1, :], a_sb[:, 1, :], start=False,
                     stop=True)
    nc.vector.tensor_tensor(ub_sb[:], u_ps[:], cu_sb[:], op=ALU.add)
    nc.scalar.activation(ue_sb[:], ub_sb[:], AF.Exp)
    ssum = ps_sm.tile([1, 384], F32, tag="sm")
    nc.tensor.matmul(ssum[:], ones41[:], ue_sb[:], start=True, stop=True)
    nc.scalar.copy(rec_sb[:], ssum[:])
    rb = ps_sm.tile([4, 384], F32, tag="sm")
    nc.tensor.matmul(rb[:], ones14[:], rec_sb[:], start=True, stop=True)
    rcp4 = consts.tile([4, 384], F32)
    nc.vector.reciprocal_approx_fast(rcp4[:], rb[:])
    nc.vector.tensor_tensor(u_all[:], ue_sb[:], rcp4[:], op=ALU.mult)

    # G[r, h, w] = U3[h, r] * U2[w, r] (spatial); Gc swaps h/w roles
    # (built in per-pt chunks inside the final loop to pipeline)

    # ---- MT = (Wr @ U1 diag(lam))^T  [4, 256] bf16 ----
    for kk2 in range(2):
        u1t_ps = ps_sm.tile([128, 4], F32, tag="sm")
        nc.tensor.transpose(u1t_ps[:], u_all[0:4, kk2 * 128:(kk2 + 1) * 128],
                            ident[0:4, 0:4])
        nc.scalar.copy(u1n[:, kk2, :], u1t_ps[:])
    mt_ps = ps_sm.tile([4, 256], F32, tag="sm")
    nc.tensor.matmul(mt_ps[:], u1n[:, 0, :], wrt_sb[:, 0, :], start=True,
                     stop=False)
    nc.tensor.matmul(mt_ps[:], u1n[:, 1, :], wrt_sb[:, 1, :], start=False,
                     stop=True)
    nc.vector.tensor_scalar(MT[:], mt_ps[:], lam_sb[:], None, op0=ALU.mult)

    # ---- spectral attention ----
    f_ps = ps_sm.tile([128, 2, 128], F32, tag="sm")
    for ct in range(2):
        nc.tensor.matmul(f_ps[:, ct, :],
                         u_all[0:4, ct * 128:(ct + 1) * 128],
                         u_all[:, 256:384], start=True, stop=True)
        nc.vector.tensor_reduce(gag[:, ct:ct + 1], f_ps[:, ct, :],
                                axis=AX.X, op=ALU.add)
        nc.vector.tensor_reduce(gag[:, 2 + ct:3 + ct], f_ps[:, ct, :],
                                axis=AX.X, op=ALU.max)
    spv = ps_sm.tile([128, 2], F32, tag="sm")
    for mm in range(2):
        for kk2 in range(4):
            nc.tensor.matmul(spv[:, mm:mm + 1], wsc_sb[:, kk2, mm, :],
                             gag[:, kk2:kk2 + 1], start=(kk2 == 0),
                             stop=(kk2 == 3))
    for mm in range(2):
        stmp = ew.tile([128, 1], F32, tag="stmp")
        nc.scalar.activation(stmp[:], spv[:, mm:mm + 1], AF.Sigmoid,
                             bias=bsc_sb[:, mm:mm + 1])
        nc.scalar.activation(spec[:, mm:mm + 1], stmp[:], AF.Sigmoid)
    nc.vector.tensor_scalar(spcA[:], spec[:], float(-alpha), None,
                            op0=ALU.mult)
    # brs row: (br * spectral) as a [1, 256] bf16 row for the rank-1 term
    brs = ew.tile([128, 2], F32, tag="brs")
    nc.vector.tensor_tensor(brs[:], br_sb[:], spec[:], op=ALU.mult)
    for ct in range(2):
        brt = ps_sm.tile([1, 128], F32, tag="sm")
        nc.tensor.transpose(brt[:], brs[:, ct:ct + 1], ident[:])
        nc.vector.tensor_copy(brsr[0:1, ct * 128:(ct + 1) * 128], brt[:])
        sat = ps_sm.tile([1, 128], F32, tag="sm")
        nc.tensor.transpose(sat[:], spcA[:, ct:ct + 1], ident[:])
        nc.vector.tensor_copy(spcAb[0:1, ct * 128:(ct + 1) * 128], sat[:])

    # ---- final stage ----
    # fused  = t2 + spcA*(E' . s);  cp = spec*((MT@Gc).s + br.s) + Fm
    for pt in range(8):
        sl = slice(pt * 512, (pt + 1) * 512)
        hs = slice(pt * 8, pt * 8 + 8)
        nc.vector.tensor_tensor(
            G[:, sl].rearrange("p (h w) -> p h w", h=8),
            u_all[:, 320:384][:, hs][:, :, None].broadcast_to([4, 8, 64]),
            u_all[:, 256:320][:, None, :].broadcast_to([4, 8, 64]),
            op=ALU.mult)
        srow_ps = ps_fin.tile([4, 512], F32, tag="sb2")
        nc.tensor.matmul(srow_ps[:], ones44b[:], G[:, sl], start=True,
                         stop=True)
        nc.scalar.activation(s4[:, sl], srow_ps[:], AF.Sigmoid,
                             scale=float(ws), bias=float(bs))
        nc.vector.tensor_tensor(
            Gc[:, sl].rearrange("p (h w) -> p h w", h=8),
            u_all[:, 256:320][:, hs][:, :, None].broadcast_to([4, 8, 64]),
            u_all[:, 320:384][:, None, :].broadcast_to([4, 8, 64]),
            op=ALU.mult)
        nc.vector.tensor_tensor(Gs[:, sl], Gc[:, sl], s4[:, sl],
                                op=ALU.mult)
        wts = []
        for ct in range(2):
            wt_ps = ps_fin.tile([128, 512], F32, tag="sb2")
            nc.tensor.matmul(wt_ps[:],
                             spcAb[0:1, ct * 128:(ct + 1) * 128],
                             s4[0:1, sl], start=True, stop=True)
            wts.append(wt_ps)
        rcs = []
        for ct in range(2):
            rc_ps = ps_conv.tile([128, 512], F32, tag="conv")
            nc.tensor.matmul(rc_ps[:], MT[:, ct * 128:(ct + 1) * 128],
                             Gs[:, sl], start=True, stop=False)
            nc.tensor.matmul(rc_ps[:], brsr[0:1, ct * 128:(ct + 1) * 128],
                             s4[0:1, sl], start=False, stop=True)
            rcs.append(rc_ps)
        for ct in range(2):
            fa = ew.tile([128, 512], BF16, tag="fa")
            nc.vector.tensor_tensor(fa[:], e_sb[:, ct, pt, :], wts[ct][:],
                                    op=ALU.mult)
            fu = outr.tile([128, 8, 64], BF16, tag="fu")
            nc.vector.tensor_tensor(
                fu[:], fa[:].rearrange("p (h w) -> p h w", h=8),
                t2_sb[:, ct, pt].rearrange("p (h w) -> p h w", h=8),
                op=ALU.add)
            nc.sync.dma_start(fused_o[:, ct, pt * 8:(pt + 1) * 8, :], fu[:])
        for ct in range(2):
            cp = outr.tile([128, 8, 64], BF16, tag="cp")
            nc.vector.scalar_tensor_tensor(
                cp[:], rcs[ct][:].rearrange("p (h w) -> p h w", h=8),
                spec[:, ct:ct + 1],
                fm[:, ct, pt].rearrange("p (h w) -> p h w", h=8),
                op0=ALU.mult, op1=ALU.add)
            nc.sync.dma_start(cpr_o[:, ct, pt * 8:(pt + 1) * 8, :], cp[:])
    ctx.close()


def _prep_weights(W3, b3, Wa1, ba1, Wa2, ba2, Wa3, ba3, Wu, bu, Wr, br,
                  Wsa, bsa, Wsm, bsm):
    f = np.float32
    # w3t[p, kt, ct, t, co] = W3[ct*128+co, kt*128+p, dy, dx]
    w3t = np.ascontiguousarray(
        W3.reshape(2, 128, 4, 128, 9).transpose(3, 2, 0, 4, 1)).astype(BF)
    b3h = np.ascontiguousarray(b3.reshape(2, 128).T).astype(f)
    # wa[0, s, m, o] = Wa_m[o, s]; avg column scaled by 1/pool_n
    was = []
    for m, wv in enumerate((Wa1, Wa2, Wa3)):
        wv = np.array(wv, f).copy()
        wv[:, 0] /= (HW if m == 0 else C * H)
        was.append(wv)
    wa = np.ascontiguousarray(
        np.stack(was, axis=0).transpose(2, 0, 1)[None]).astype(f)
    # wu[p, ct, k] = Wu[k, ct*128+p]
    wu = np.ascontiguousarray(
        Wu.reshape(K, 2, 128).transpose(2, 1, 0)).astype(f)
    # cu[k, n] = (Wu @ ba_m + bu)[k] for n in mode-m block
    cus = [Wu @ bam + bu for bam in (ba1, ba2, ba3)]
    cu = np.concatenate([np.tile(cus[0][:, None], (1, 256)),
                         np.tile(cus[1][:, None], (1, 64)),
                         np.tile(cus[2][:, None], (1, 64))], axis=1)
    cu = np.ascontiguousarray(cu).astype(f)
    # wrt[p, kk, m] = Wr[m, kk*128+p]
    wrt = np.ascontiguousarray(
        Wr.reshape(256, 2, 128).transpose(2, 1, 0)).astype(f)
    brh = np.ascontiguousarray(br.reshape(2, 128).T).astype(f)
    # wsc[p, kk, mm, m]: kk<2 -> Wsa/(W+H) (mean folded), kk>=2 -> Wsm
    wsa_r = (Wsa / 128.0).reshape(2, 128, 2, 128).transpose(3, 2, 0, 1)
    wsm_r = Wsm.reshape(2, 128, 2, 128).transpose(3, 2, 0, 1)
    wsc = np.ascontiguousarray(
        np.concatenate([wsa_r, wsm_r], axis=1)).astype(f)
    bsc = np.ascontiguousarray((bsa + bsm).reshape(2, 128).T).astype(f)
    return dict(w3t=w3t, b3=b3h, wa=wa, wu=wu, cu=cu, wrt=wrt, br=brh,
                wsc=wsc, bsc=bsc)


_CACHE = {}


def kernel(frm_feat, other_feat, W3, b3, Wa1, ba1, Wa2, ba2, Wa3, ba3,
           Wu, bu, Wr, br, ws, bs, Wsa, bsa, Wsm, bsm, alpha, lam,
           _trace=False, _tmpdir=None):
    key = (float(alpha), float(ws), float(bs))
    if key not in _CACHE:
        _CACHE[key] = build_program(float(alpha), float(ws), float(bs))
    nc = _CACHE[key]

    wd = _prep_weights(np.asarray(W3, np.float32), np.asarray(b3),
                       np.asarray(Wa1), np.asarray(ba1), np.asarray(Wa2),
                       np.asarray(ba2), np.asarray(Wa3), np.asarray(ba3),
                       np.asarray(Wu), np.asarray(bu),
                       np.asarray(Wr, np.float32), np.asarray(br),
                       np.asarray(Wsa, np.float32), np.asarray(bsa),
                       np.asarray(Wsm, np.float32), np.asarray(bsm))
    wd["lam"] = np.asarray(lam, np.float32).reshape(4, 1)

    in_maps = []
    for b_i in range(NCORES):
        m = dict(wd)
        m["frm"] = frm_bat(frm_feat, b_i)
        m["oth"] = frm_bat(other_feat, b_i)
        in_maps.append(m)

    res = bass_utils.run_bass_kernel_spmd(
        nc, in_maps, core_ids=list(range(NCORES)), trace=_trace,
        tmpdir=_tmpdir)
    fused = np.stack([_unshard(res.results[i]["fused"])
                      for i in range(NCORES)])
    cpr = np.stack([_unshard(res.results[i]["cpr"])
                    for i in range(NCORES)])
    kernel._last_exec_time_ns = res.exec_time_ns
    kernel._last_results = res
    return fused, cpr


def frm_bat(x, b_i):
    """[B, 256, H, W] f32 -> [128, 2, H, W] bf16 for batch b_i."""
    return np.ascontiguousarray(
        np.asarray(x[b_i], np.float32).reshape(2, 128, H, W)
        .transpose(1, 0, 2, 3)).astype(BF)


def _unshard(a):
    """[128, 2, H, W] bf16 -> [256, H, W] f32."""
    return np.ascontiguousarray(
        np.asarray(a).transpose(1, 0, 2, 3)).reshape(256, H, W)\
        .astype(np.float32)


# revision 44
# speedup vs baseline: 1.0276x; 1.0227x over previous
"""Trainium2 Bass kernel for nn_MDRMWithCPRecon (optimized v2).

Sharding: pure data parallel over batch B=8 -> one batch element per
NeuronCore. All parameters replicated.

Per-core pipeline:
  x  = cat(frm, oth)                    [512, 64, 64]
  Fm = lrelu(conv3x3(x, W3) + b3)      [256, 64, 64]   <- bulk of FLOPs
  U1/U2/U3 rank-4 softmax factors from pooled stats
  spatial  = sigmoid(ws * U3 @ U2^T + bs)
  spectral = sigmoid(sigmoid(Wsa@mean + Wsm@max + biases))
  Wt = spectral x spatial
  fused    = a*Wt*frm + (1-a)*(1-Wt)*oth
  cp_recon = (Wr @ cp + br) * Wt + Fm,  cp = rank-4 CP(U1,U2,U3,lam)

v2 changes vs baseline (294us):
  - whole conv in bf16 (tolerance 2e-2; measured err stays ~1e-3):
    halves input DMA and SBUF, same 1 cycle/row PE rate as f32r.
  - inputs DMA'd straight into the padded SBUF image (bitcast view),
    no staging tiles / tensor_copy casts; borders memset once.
  - DMA order interleaves per-kt weight and image chunks so the PE can
    start accumulating ~2us in and is fed at matched rate.
  - Fm stays in SBUF as bf16 (kills the 8MB fm_scratch DRAM roundtrip).
  - E' = ((1-a)/a)*oth - frm precomputed on Vector during the conv.
  - pooled-stats -> U1/U2/U3 softmax chain batched into one [4, 384]
    tile (one exp, one ones-matmul row-sum, one reciprocal, one
    broadcast) instead of ~90 tiny serialized ops.
  - final elementwise stage in bf16 (2x DVE), spread over
    Vector/Scalar/GpSimd; outputs stored bf16, cast to f32 on host.
"""

import numpy as np
import ml_dtypes

import concourse.bacc as bacc
import concourse.bass as bass
import concourse.tile as tile
from concourse import mybir, bass_utils

F32 = mybir.dt.float32
BF16 = mybir.dt.bfloat16
AF = mybir.ActivationFunctionType
ALU = mybir.AluOpType
AX = mybir.AxisListType

B, C, H, W, K = 8, 256, 64, 64, 4
HW = H * W
NCORES = 8
BF = ml_dtypes.bfloat16

# tap order: center tap first and a full-window tap last (PSUM start/stop)
TAPS = [(1, 1), (0, 0), (0, 2), (2, 0), (2, 2), (0, 1), (1, 2), (1, 0),
        (2, 1)]


def build_program(alpha, ws, bs):
    from concourse.masks import make_identity

    nc = bacc.Bacc("TRN2", target_bir_lowering=False, debug=False,
                   num_devices=NCORES)

    # ---- DRAM I/O (per core) ----
    frm_d = nc.dram_tensor("frm", [128, 2, H, W], BF16, kind="ExternalInput")
    oth_d = nc.dram_tensor("oth", [128, 2, H, W], BF16, kind="ExternalInput")
    w3t_d = nc.dram_tensor("w3t", [128, 4, 2, 9, 128], BF16,
                           kind="ExternalInput")
    b3_d = nc.dram_tensor("b3", [128, 2], F32, kind="ExternalInput")
    wa_d = nc.dram_tensor("wa", [1, 2, 3, 256], F32, kind="ExternalInput")
    wu_d = nc.dram_tensor("wu", [128, 2, 4], F32, kind="ExternalInput")
    cu_d = nc.dram_tensor("cu", [4, 384], F32, kind="ExternalInput")
    wrt_d = nc.dram_tensor("wrt", [128, 2, 256], F32, kind="ExternalInput")
    br_d = nc.dram_tensor("br", [128, 2], F32, kind="ExternalInput")
    wsc_d = nc.dram_tensor("wsc", [128, 4, 2, 128], F32, kind="ExternalInput")
    bsc_d = nc.dram_tensor("bsc", [128, 2], F32, kind="ExternalInput")
    lam_d = nc.dram_tensor("lam", [4, 1], F32, kind="ExternalInput")
    fused_o = nc.dram_tensor("fused", [128, 2, H, W], BF16,
                             kind="ExternalOutput")
    cpr_o = nc.dram_tensor("cpr", [128, 2, H, W], BF16,
                           kind="ExternalOutput")

    with tile.TileContext(nc) as tc:
        _build_tile(tc, nc, make_identity, locals(), alpha, ws, bs)
    nc.compile()
    return nc


def _build_tile(tc, nc, make_identity, T, alpha, ws, bs):
    frm_d, oth_d, w3t_d, b3_d = T["frm_d"], T["oth_d"], T["w3t_d"], T["b3_d"]
    wa_d, wu_d, cu_d, wrt_d = T["wa_d"], T["wu_d"], T["cu_d"], T["wrt_d"]
    br_d, wsc_d, bsc_d, lam_d = T["br_d"], T["wsc_d"], T["bsc_d"], T["lam_d"]
    fused_o, cpr_o = T["fused_o"], T["cpr_o"]

    import contextlib
    ctx = contextlib.ExitStack()
    consts = ctx.enter_context(tc.tile_pool(name="consts", bufs=1))
    ew = ctx.enter_context(tc.tile_pool(name="ew", bufs=3))
    outr = ctx.enter_context(tc.tile_pool(name="outr", bufs=3))
    ps_conv = ctx.enter_context(tc.tile_pool(name="ps_conv", bufs=2,
                                             space="PSUM"))
    ps_sm = ctx.enter_context(tc.tile_pool(name="ps_sm", bufs=2,
                                           space="PSUM"))
    ps_fin = ctx.enter_context(tc.tile_pool(name="ps_fin", bufs=4,
                                            space="PSUM"))

    # ---- persistent SBUF tiles ----
    xin = consts.tile([128, 4, 66, 64], BF16)     # row-padded cat(frm,oth)
    w3t = consts.tile([128, 4, 2, 9, 128], BF16)  # conv weights
    fm = consts.tile([128, 2, 8, 512], BF16)      # conv output Fm
    e_sb = consts.tile([128, 2, 8, 512], BF16)    # E' = k*oth - frm
    t2_sb = consts.tile([128, 2, 8, 512], BF16)   # (1-a)*oth
    b3_sb = consts.tile([128, 2], F32)
    wa_sb = consts.tile([1, 2, 3, 256], F32)
    wu_sb = consts.tile([128, 2, 4], F32)
    cu_sb = consts.tile([4, 384], F32)
    wrt_sb = consts.tile([128, 2, 256], F32)
    br_sb = consts.tile([128, 2], F32)
    wsc_sb = consts.tile([128, 4, 2, 128], F32)
    bsc_sb = consts.tile([128, 2], F32)
    lam_sb = consts.tile([4, 1], F32)
    # stats
    sums1 = consts.tile([128, 2, 8], F32)      # per-(ct,pt) channel sums
    pp_sum = consts.tile([128, 2, 8, 64], F32)  # per-pt sum over h -> [c,w]
    pp_max = consts.tile([128, 2, 8, 64], F32)
    pq_sum = consts.tile([128, 2, 2, 64], F32)  # [m2|m3][ct] pooled sums
    pq_max = consts.tile([128, 2, 2, 64], F32)
    stat1 = consts.tile([128, 4], F32)          # [sum ct0, sum ct1, max...]
    # U chain (pooled rows kept on partition 0 only)
    row1s = consts.tile([1, 256], F32)
    row1m = consts.tile([1, 256], F32)
    row23s = consts.tile([1, 128], F32)
    row23m = consts.tile([1, 128], F32)
    a_sb = consts.tile([128, 2, 384], F32)
    ub_sb = consts.tile([4, 384], F32)
    ue_sb = consts.tile([4, 384], F32)
    rec_sb = consts.tile([1, 384], F32)
    u_all = consts.tile([4, 384], F32)
    u1n = consts.tile([128, 2, 4], F32)
    gag = consts.tile([128, 4], F32)
    spec = consts.tile([128, 2], F32)
    spcA = consts.tile([128, 2], F32)
    mx2 = consts.tile([128, 128], F32)
    G = consts.tile([4, HW], BF16)    # spatial: U3[h]*U2[w]
    Gc = consts.tile([4, HW], BF16)   # cp recon: U2[h]*U3[w]
    Gs = consts.tile([4, HW], BF16)   # Gc * s  (spatial sigmoid folded)
    s4 = consts.tile([4, HW], BF16)   # sigmoid spatial map, 4 rows
    MT = consts.tile([4, 256], BF16)
    brsr = consts.tile([1, 256], BF16)
    spcAb = consts.tile([1, 256], BF16)

    ident = consts.tile([128, 128], F32)
    make_identity(nc, ident[:])
    ones128 = consts.tile([128, 1], F32)
    nc.gpsimd.memset(ones128[:], 1.0)
    ones41 = consts.tile([4, 1], F32)
    nc.gpsimd.memset(ones41[:], 1.0)
    ones14 = consts.tile([1, 4], F32)
    nc.gpsimd.memset(ones14[:], 1.0)
    ones44b = consts.tile([4, 4], BF16)
    nc.gpsimd.memset(ones44b[:], 1.0)
    ones1x128b = consts.tile([1, 128], BF16)
    nc.gpsimd.memset(ones1x128b[:], 1.0)

    # ---- zero the padded top/bottom rows (cols handled per-tap) ----
    nc.gpsimd.memset(xin[:, :, 0:1, :], 0.0)
    nc.gpsimd.memset(xin[:, :, 65:66, :], 0.0)

    # ---- DMA order: b3 first, then per-kt (weights ct0, image h0) pairs,
    # then ct1 weights, h1 image halves, then smalls ----
    srcs = [frm_d, frm_d, oth_d, oth_d]
    for kt in range(4):
        nc.scalar.dma_start(w3t[:, kt, 0], w3t_d[:, kt, 0])
        nc.sync.dma_start(xin[:, kt, 1:33, :],
                          srcs[kt][:, kt % 2, 0:32, :])
        if kt == 0:
            nc.scalar.dma_start(b3_sb[:], b3_d[:])
    for kt in range(4):
        nc.scalar.dma_start(w3t[:, kt, 1], w3t_d[:, kt, 1])
    for kt in range(4):
        nc.sync.dma_start(xin[:, kt, 33:65, :],
                          srcs[kt][:, kt % 2, 32:64, :])
    for sb, dd in ((wa_sb, wa_d), (wu_sb, wu_d), (cu_sb, cu_d),
                   (wrt_sb, wrt_d), (br_sb, br_d), (wsc_sb, wsc_d),
                   (bsc_sb, bsc_d), (lam_sb, lam_d)):
        nc.scalar.dma_start(sb[:], dd[:])

    kk_e = float((1.0 - alpha) / alpha)

    # ---- conv3x3 + lrelu + streaming stats ----
    for pt in range(8):
        for ct in range(2):
            ps = ps_conv.tile([128, 8, 64], F32, tag="conv")
            idx = 0
            for kt in range(4):
                for (dy, dx) in TAPS:
                    # column edge handling: dx=0 drops out-col 0, dx=2
                    # drops out-col 63 (zero contribution at the border)
                    co0, co1 = (1, 64) if dx == 0 else (0, 63) \
                        if dx == 2 else (0, 64)
                    ci0 = dx - 1 + co0
                    nc.tensor.matmul(
                        ps[:, :, co0:co1],
                        w3t[:, kt, ct, dy * 3 + dx, :],
                        xin[:, kt, pt * 8 + dy: pt * 8 + dy + 8,
                            ci0: ci0 + co1 - co0],
                        start=(idx == 0), stop=(idx == 35))
                    idx += 1
            nc.scalar.activation(fm[:, ct, pt].rearrange(
                "p (h w) -> p h w", h=8), ps[:], AF.Lrelu,
                bias=b3_sb[:, ct:ct + 1], alpha=0.01,
                accum_out=sums1[:, ct, pt:pt + 1])
            blk = fm[:, ct, pt].rearrange("p (h w) -> p h w", h=8)
            blk_t = fm[:, ct, pt].rearrange("p (h w) -> p w h", h=8)
            # mode3 (per-h) stats: disjoint slices, written directly
            nc.vector.tensor_reduce(pq_sum[:, 1, ct, pt * 8:(pt + 1) * 8],
                                    blk, axis=AX.X, op=ALU.add)
            nc.vector.tensor_reduce(pq_max[:, 1, ct, pt * 8:(pt + 1) * 8],
                                    blk, axis=AX.X, op=ALU.max)
            # mode2 (per-w) partials, combined after the loop
            nc.vector.tensor_reduce(pp_sum[:, ct, pt, :], blk_t,
                                    axis=AX.X, op=ALU.add)
            nc.vector.tensor_reduce(pp_max[:, ct, pt, :], blk_t,
                                    axis=AX.X, op=ALU.max)
            # E'/t2 precompute (no conv dependency; fills idle slots).
            # pt7's are deferred past the stats combine (critical path).
            if pt < 7:
                nc.vector.scalar_tensor_tensor(
                    e_sb[:, ct, pt].rearrange("p (h w) -> p h w", h=8),
                    xin[:, 2 + ct, 1 + pt * 8: 9 + pt * 8, :], kk_e,
                    xin[:, ct, 1 + pt * 8: 9 + pt * 8, :],
                    op0=ALU.mult, op1=ALU.subtract)
                nc.vector.tensor_scalar(
                    t2_sb[:, ct, pt].rearrange("p (h w) -> p h w", h=8),
                    xin[:, 2 + ct, 1 + pt * 8: 9 + pt * 8, :],
                    float(1.0 - alpha), None, op0=ALU.mult)

    # ---- combine stats (global max = max over h of per-h maxes) ----
    nc.vector.tensor_reduce(stat1[:, 0:2], sums1[:], axis=AX.X, op=ALU.add)
    nc.vector.tensor_reduce(stat1[:, 2:4], pq_max[:, 1], axis=AX.X,
                            op=ALU.max)
    nc.vector.tensor_reduce(
        pq_sum[:, 0], pp_sum[:].rearrange("p c t w -> p c w t"),
        axis=AX.X, op=ALU.add)
    nc.vector.tensor_reduce(
        pq_max[:, 0], pp_max[:].rearrange("p c t w -> p c w t"),
        axis=AX.X, op=ALU.max)

    # mode1 rows via per-column transposes; avg-scales folded into wa
    rowdst = [(row1s, 0), (row1s, 128), (row1m, 0), (row1m, 128)]
    for j, (dst, off) in enumerate(rowdst):
        tpj = ps_sm.tile([1, 128], F32, tag="sm")
        nc.tensor.transpose(tpj[:], stat1[:, j:j + 1], ident[:])
        nc.scalar.copy(dst[0:1, off:off + 128], tpj[:])

    # mode2/3 sum rows: ones-matmul over channels, add ct halves
    srow = ps_sm.tile([1, 2, 2, 64], F32, tag="sm")
    nc.tensor.matmul(srow[:].rearrange("p a b c -> p (a b c)"), ones128[:],
                     pq_sum[:].rearrange("p a b c -> p (a b c)"),
                     start=True, stop=True)
    sr_sb = consts.tile([1, 2, 2, 64], F32)
    nc.scalar.copy(sr_sb[:], srow[:])
    nc.vector.tensor_tensor(row23s[0:1, :].rearrange("p (m w) -> p m w",
                                                     m=2),
                            sr_sb[:, :, 0, :], sr_sb[:, :, 1, :], op=ALU.add)
    # mode2/3 max rows: ct-combine, transpose, reduce, transpose back
    nc.vector.tensor_tensor(mx2[:].rearrange("p (m w) -> p m w", m=2),
                            pq_max[:, :, 0, :], pq_max[:, :, 1, :],
                            op=ALU.max)
    mxT = ps_sm.tile([128, 128], F32, tag="sm")
    nc.tensor.transpose(mxT[:], mx2[:], ident[:])
    mcol = ew.tile([128, 1], F32, tag="mcol")
    nc.vector.tensor_reduce(mcol[:], mxT[:], axis=AX.X, op=ALU.max)
    mrow = ps_sm.tile([1, 128], F32, tag="sm")
    nc.tensor.transpose(mrow[:], mcol[:], ident[:])
    nc.scalar.copy(row23m[:], mrow[:])

    # ---- a[o, n] = sum_s wa_s[o] * row_s[n]  (rank-1 outer products) ----
    for ct in range(2):
        ap_t = ps_sm.tile([128, 384], F32, tag="sm")
        cs = ct * 128
        nc.tensor.matmul(ap_t[:, 0:256], wa_sb[0:1, 0, 0, cs:cs + 128],
                         row1s[:], start=True, stop=False)
        nc.tensor.matmul(ap_t[:, 0:256], wa_sb[0:1, 1, 0, cs:cs + 128],
                         row1m[:], start=False, stop=True)
        nc.tensor.matmul(ap_t[:, 256:320], wa_sb[0:1, 0, 1, cs:cs + 128],
                         row23s[:, 0:64], start=True, stop=False)
        nc.tensor.matmul(ap_t[:, 256:320], wa_sb[0:1, 1, 1, cs:cs + 128],
                         row23m[:, 0:64], start=False, stop=True)
        nc.tensor.matmul(ap_t[:, 320:384], wa_sb[0:1, 0, 2, cs:cs + 128],
                         row23s[:, 64:128], start=True, stop=False)
        nc.tensor.matmul(ap_t[:, 320:384], wa_sb[0:1, 1, 2, cs:cs + 128],
                         row23m[:, 64:128], start=False, stop=True)
        nc.scalar.copy(a_sb[:, ct, :], ap_t[:])

    # ---- u = Wu @ a + (Wu@ba + bu)  [4, 384], then softmax over k ----
    u_ps = ps_sm.tile([4, 384], F32, tag="sm")
    nc.tensor.matmul(u_ps[:], wu_sb[:, 0, :], a_sb[:, 0, :], start=True,
                     stop=False)
    nc.tensor.matmul(u_ps[:], wu_sb[:, 1, :], a_sb[:, 1, :], start=False,
                     stop=True)
    nc.vector.tensor_tensor(ub_sb[:], u_ps[:], cu_sb[:], op=ALU.add)
    nc.scalar.activation(ue_sb[:], ub_sb[:], AF.Exp)
    ssum = ps_sm.tile([1, 384], F32, tag="sm")
    nc.tensor.matmul(ssum[:], ones41[:], ue_sb[:], start=True, stop=True)
    nc.scalar.copy(rec_sb[:], ssum[:])
    rb = ps_sm.tile([4, 384], F32, tag="sm")
    nc.tensor.matmul(rb[:], ones14[:], rec_sb[:], start=True, stop=True)
    rcp4 = consts.tile([4, 384], F32)
    nc.vector.reciprocal_approx_fast(rcp4[:], rb[:])
    nc.vector.tensor_tensor(u_all[:], ue_sb[:], rcp4[:], op=ALU.mult)

    # G[r, h, w] = U3[h, r] * U2[w, r] (spatial); Gc swaps h/w roles
    # (built in per-pt chunks inside the final loop to pipeline)

    # ---- MT = (Wr @ U1 diag(lam))^T  [4, 256] bf16 ----
    for kk2 in range(2):
        u1t_ps = ps_sm.tile([128, 4], F32, tag="sm")
        nc.tensor.transpose(u1t_ps[:], u_all[0:4, kk2 * 128:(kk2 + 1) * 128],
                            ident[0:4, 0:4])
        nc.scalar.copy(u1n[:, kk2, :], u1t_ps[:])
    mt_ps = ps_sm.tile([4, 256], F32, tag="sm")
    nc.tensor.matmul(mt_ps[:], u1n[:, 0, :], wrt_sb[:, 0, :], start=True,
                     stop=False)
    nc.tensor.matmul(mt_ps[:], u1n[:, 1, :], wrt_sb[:, 1, :], start=False,
                     stop=True)
    nc.vector.tensor_scalar(MT[:], mt_ps[:], lam_sb[:], None, op0=ALU.mult)

    # ---- spectral attention ----
    f_ps = ps_sm.tile([128, 2, 128], F32, tag="sm")
    for ct in range(2):
        nc.tensor.matmul(f_ps[:, ct, :],
                         u_all[0:4, ct * 128:(ct + 1) * 128],
                         u_all[:, 256:384], start=True, stop=True)
        nc.vector.tensor_reduce(gag[:, ct:ct + 1], f_ps[:, ct, :],
                                axis=AX.X, op=ALU.add)
        nc.vector.tensor_reduce(gag[:, 2 + ct:3 + ct], f_ps[:, ct, :],
                                axis=AX.X, op=ALU.max)
    spv = ps_sm.tile([128, 2], F32, tag="sm")
    for mm in range(2):
        for kk2 in range(4):
            nc.tensor.matmul(spv[:, mm:mm + 1], wsc_sb[:, kk2, mm, :],
                             gag[:, kk2:kk2 + 1], start=(kk2 == 0),
                             stop=(kk2 == 3))
    for mm in range(2):
        stmp = ew.tile([128, 1], F32, tag="stmp")
        nc.scalar.activation(stmp[:], spv[:, mm:mm + 1], AF.Sigmoid,
                             bias=bsc_sb[:, mm:mm + 1])
        nc.scalar.activation(spec[:, mm:mm + 1], stmp[:], AF.Sigmoid)
    nc.vector.tensor_scalar(spcA[:], spec[:], float(-alpha), None,
                            op0=ALU.mult)
    # brs row: (br * spectral) as a [1, 256] bf16 row for the rank-1 term
    brs = ew.tile([128, 2], F32, tag="brs")
    nc.vector.tensor_tensor(brs[:], br_sb[:], spec[:], op=ALU.mult)
    for ct in range(2):
        brt = ps_sm.tile([1, 128], F32, tag="sm")
        nc.tensor.transpose(brt[:], brs[:, ct:ct + 1], ident[:])
        nc.vector.tensor_copy(brsr[0:1, ct * 128:(ct + 1) * 128], brt[:])
        sat = ps_sm.tile([1, 128], F32, tag="sm")
        nc.tensor.transpose(sat[:], spcA[:, ct:ct + 1], ident[:])
        nc.vector.tensor_copy(spcAb[0:1, ct * 128:(ct + 1) * 128], sat[:])
    for ct in range(2):  # deferred pt7 E'/t2
        nc.vector.scalar_tensor_tensor(
            e_sb[:, ct, 7].rearrange("p (h w) -> p h w", h=8),
            xin[:, 2 + ct, 57:65, :], kk_e, xin[:, ct, 57:65, :],
            op0=ALU.mult, op1=ALU.subtract)
        nc.vector.tensor_scalar(
            t2_sb[:, ct, 7].rearrange("p (h w) -> p h w", h=8),
            xin[:, 2 + ct, 57:65, :], float(1.0 - alpha), None,
            op0=ALU.mult)

    # ---- final stage ----
    # fused  = t2 + spcA*(E' . s);  cp = spec*((MT@Gc).s + br.s) + Fm
    # loop 1: spatial map chunks + sigmoid (no act-table thrash)
    for pt in range(8):
        sl = slice(pt * 512, (pt + 1) * 512)
        hs = slice(pt * 8, pt * 8 + 8)
        nc.vector.tensor_tensor(
            G[:, sl].rearrange("p (h w) -> p h w", h=8),
            u_all[:, 320:384][:, hs][:, :, None].broadcast_to([4, 8, 64]),
            u_all[:, 256:320][:, None, :].broadcast_to([4, 8, 64]),
            op=ALU.mult)
        srow_ps = ps_fin.tile([4, 512], F32, tag="sb2")
        nc.tensor.matmul(srow_ps[:], ones44b[:], G[:, sl], start=True,
                         stop=True)
        nc.scalar.activation(s4[:, sl], srow_ps[:], AF.Sigmoid,
                             scale=float(ws), bias=float(bs))
        nc.vector.tensor_tensor(
            Gc[:, sl].rearrange("p (h w) -> p h w", h=8),
            u_all[:, 256:320][:, hs][:, :, None].broadcast_to([4, 8, 64]),
            u_all[:, 320:384][:, None, :].broadcast_to([4, 8, 64]),
            op=ALU.mult)
    # loop 2: fused/recon outputs
    for pt in range(8):
        sl = slice(pt * 512, (pt + 1) * 512)
        nc.vector.tensor_tensor(Gs[:, sl], Gc[:, sl], s4[:, sl],
                                op=ALU.mult)
        wts = []
        for ct in range(2):
            wt_ps = ps_fin.tile([128, 512], F32, tag="sb2")
            nc.tensor.matmul(wt_ps[:],
                             spcAb[0:1, ct * 128:(ct + 1) * 128],
                             s4[0:1, sl], start=True, stop=True)
            wt_sb = ew.tile([128, 512], BF16, tag="wtsb")
            nc.scalar.copy(wt_sb[:], wt_ps[:])
            wts.append(wt_sb)
        rcs = []
        for ct in range(2):
            rc_ps = ps_conv.tile([128, 512], F32, tag="conv")
            nc.tensor.matmul(rc_ps[:], MT[:, ct * 128:(ct + 1) * 128],
                             Gs[:, sl], start=True, stop=False)
            nc.tensor.matmul(rc_ps[:], brsr[0:1, ct * 128:(ct + 1) * 128],
                             s4[0:1, sl], start=False, stop=True)
            rcs.append(rc_ps)
        for ct in range(2):
            fa = ew.tile([128, 512], BF16, tag="fa")
            nc.vector.tensor_tensor(fa[:], e_sb[:, ct, pt, :],
                                    wts[ct][:], op=ALU.mult)
            fu = outr.tile([128, 8, 64], BF16, tag="fu")
            nc.vector.tensor_tensor(
                fu[:], fa[:].rearrange("p (h w) -> p h w", h=8),
                t2_sb[:, ct, pt].rearrange("p (h w) -> p h w", h=8),
                op=ALU.add)
            nc.sync.dma_start(fused_o[:, ct, pt * 8:(pt + 1) * 8, :], fu[:])
        for ct in range(2):
            cp = outr.tile([128, 8, 64], BF16, tag="cp")
            nc.vector.scalar_tensor_tensor(
                cp[:], rcs[ct][:].rearrange("p (h w) -> p h w", h=8),
                spec[:, ct:ct + 1],
                fm[:, ct, pt].rearrange("p (h w) -> p h w", h=8),
                op0=ALU.mult, op1=ALU.add)
            nc.sync.dma_start(cpr_o[:, ct, pt * 8:(pt + 1) * 8, :], cp[:])
    ctx.close()


def _prep_weights(W3, b3, Wa1, ba1, Wa2, ba2, Wa3, ba3, Wu, bu, Wr, br,
                  Wsa, bsa, Wsm, bsm):
    f = np.float32
    # w3t[p, kt, ct, t, co] = W3[ct*128+co, kt*128+p, dy, dx]
    w3t = np.ascontiguousarray(
        W3.reshape(2, 128, 4, 128, 9).transpose(3, 2, 0, 4, 1)).astype(BF)
    b3h = np.ascontiguousarray(b3.reshape(2, 128).T).astype(f)
    # wa[0, s, m, o] = Wa_m[o, s]; avg column scaled by 1/pool_n
    was = []
    for m, wv in enumerate((Wa1, Wa2, Wa3)):
        wv = np.array(wv, f).copy()
        wv[:, 0] /= (HW if m == 0 else C * H)
        was.append(wv)
    wa = np.ascontiguousarray(
        np.stack(was, axis=0).transpose(2, 0, 1)[None]).astype(f)
    # wu[p, ct, k] = Wu[k, ct*128+p]
    wu = np.ascontiguousarray(
        Wu.reshape(K, 2, 128).transpose(2, 1, 0)).astype(f)
    # cu[k, n] = (Wu @ ba_m + bu)[k] for n in mode-m block
    cus = [Wu @ bam + bu for bam in (ba1, ba2, ba3)]
    cu = np.concatenate([np.tile(cus[0][:, None], (1, 256)),
                         np.tile(cus[1][:, None], (1, 64)),
                         np.tile(cus[2][:, None], (1, 64))], axis=1)
    cu = np.ascontiguousarray(cu).astype(f)
    # wrt[p, kk, m] = Wr[m, kk*128+p]
    wrt = np.ascontiguousarray(
        Wr.reshape(256, 2, 128).transpose(2, 1, 0)).astype(f)
    brh = np.ascontiguousarray(br.reshape(2, 128).T).astype(f)
    # wsc[p, kk, mm, m]: kk<2 -> Wsa/(W+H) (mean folded), kk>=2 -> Wsm
    wsa_r = (Wsa / 128.0).reshape(2, 128, 2, 128).transpose(3, 2, 0, 1)
    wsm_r = Wsm.reshape(2, 128, 2, 128).transpose(3, 2, 0, 1)
    wsc = np.ascontiguousarray(
        np.concatenate([wsa_r, wsm_r], axis=1)).astype(f)
    bsc = np.ascontiguousarray((bsa + bsm).reshape(2, 128).T).astype(f)
    return dict(w3t=w3t, b3=b3h, wa=wa, wu=wu, cu=cu, wrt=wrt, br=brh,
                wsc=wsc, bsc=bsc)


_CACHE = {}


def kernel(frm_feat, other_feat, W3, b3, Wa1, ba1, Wa2, ba2, Wa3, ba3,
           Wu, bu, Wr, br, ws, bs, Wsa, bsa, Wsm, bsm, alpha, lam,
           _trace=False, _tmpdir=None):
    key = (float(alpha), float(ws), float(bs))
    if key not in _CACHE:
        _CACHE[key] = build_program(float(alpha), float(ws), float(bs))
    nc = _CACHE[key]

    wd = _prep_weights(np.asarray(W3, np.float32), np.asarray(b3),
                       np.asarray(Wa1), np.asarray(ba1), np.asarray(Wa2),
                       np.asarray(ba2), np.asarray(Wa3), np.asarray(ba3),
                       np.asarray(Wu), np.asarray(bu),
                       np.asarray(Wr, np.float32), np.asarray(br),
                       np.asarray(Wsa, np.float32), np.asarray(bsa),
                       np.asarray(Wsm, np.float32), np.asarray(bsm))
    wd["lam"] = np.asarray(lam, np.float32).reshape(4, 1)

    in_maps = []
    for b_i in range(NCORES):
        m = dict(wd)
        m["frm"] = frm_bat(frm_feat, b_i)
        m["oth"] = frm_bat(other_feat, b_i)
        in_maps.append(m)

    res = bass_utils.run_bass_kernel_spmd(
        nc, in_maps, core_ids=list(range(NCORES)), trace=_trace,
        tmpdir=_tmpdir)
    fused = np.stack([_unshard(res.results[i]["fused"])
                      for i in range(NCORES)])
    cpr = np.stack([_unshard(res.results[i]["cpr"])
                    for i in range(NCORES)])
    kernel._last_exec_time_ns = res.exec_time_ns
    kernel._last_results = res
    return fused, cpr


def frm_bat(x, b_i):
    """[B, 256, H, W] f32 -> [128, 2, H, W] bf16 for batch b_i."""
    return np.ascontiguousarray(
        np.asarray(x[b_i], np.float32).reshape(2, 128, H, W)
        .transpose(1, 0, 2, 3)).astype(BF)


def _unshard(a):
    """[128, 2, H, W] bf16 -> [256, H, W] f32."""
    return np.ascontiguousarray(
        np.asarray(a).transpose(1, 0, 2, 3)).reshape(256, H, W)\
        .astype(np.float32)


# revision 48
# speedup vs baseline: 1.0353x; 1.0075x over previous
"""Trainium2 Bass kernel for nn_MDRMWithCPRecon (optimized v2).

Sharding: pure data parallel over batch B=8 -> one batch element per
NeuronCore. All parameters replicated.

Per-core pipeline:
  x  = cat(frm, oth)                    [512, 64, 64]
  Fm = lrelu(conv3x3(x, W3) + b3)      [256, 64, 64]   <- bulk of FLOPs
  U1/U2/U3 rank-4 softmax factors from pooled stats
  spatial  = sigmoid(ws * U3 @ U2^T + bs)
  spectral = sigmoid(sigmoid(Wsa@mean + Wsm@max + biases))
  Wt = spectral x spatial
  fused    = a*Wt*frm + (1-a)*(1-Wt)*oth
  cp_recon = (Wr @ cp + br) * Wt + Fm,  cp = rank-4 CP(U1,U2,U3,lam)

v2 changes vs baseline (294us):
  - whole conv in bf16 (tolerance 2e-2; measured err stays ~1e-3):
    halves input DMA and SBUF, same 1 cycle/row PE rate as f32r.
  - inputs DMA'd straight into the padded SBUF image (bitcast view),
    no staging tiles / tensor_copy casts; borders memset once.
  - DMA order interleaves per-kt weight and image chunks so the PE can
    start accumulating ~2us in and is fed at matched rate.
  - Fm stays in SBUF as bf16 (kills the 8MB fm_scratch DRAM roundtrip).
  - E' = ((1-a)/a)*oth - frm precomputed on Vector during the conv.
  - pooled-stats -> U1/U2/U3 softmax chain batched into one [4, 384]
    tile (one exp, one ones-matmul row-sum, one reciprocal, one
    broadcast) instead of ~90 tiny serialized ops.
  - final elementwise stage in bf16 (2x DVE), spread over
    Vector/Scalar/GpSimd; outputs stored bf16, cast to f32 on host.
"""

import numpy as np
import ml_dtypes

import concourse.bacc as bacc
import concourse.bass as bass
import concourse.tile as tile
from concourse import mybir, bass_utils

F32 = mybir.dt.float32
BF16 = mybir.dt.bfloat16
AF = mybir.ActivationFunctionType
ALU = mybir.AluOpType
AX = mybir.AxisListType

B, C, H, W, K = 8, 256, 64, 64, 4
HW = H * W
NCORES = 8
BF = ml_dtypes.bfloat16

# tap order: center tap first and a full-window tap last (PSUM start/stop)
TAPS = [(1, 1), (0, 0), (0, 2), (2, 0), (2, 2), (0, 1), (1, 2), (1, 0),
        (2, 1)]


def build_program(alpha, ws, bs):
    from concourse.masks import make_identity

    nc = bacc.Bacc("TRN2", target_bir_lowering=False, debug=False,
                   num_devices=NCORES)

    # ---- DRAM I/O (per core) ----
    frm_d = nc.dram_tensor("frm", [128, 2, H, W], BF16, kind="ExternalInput")
    oth_d = nc.dram_tensor("oth", [128, 2, H, W], BF16, kind="ExternalInput")
    w3t_d = nc.dram_tensor("w3t", [128, 4, 2, 9, 128], BF16,
                           kind="ExternalInput")
    b3_d = nc.dram_tensor("b3", [128, 2], F32, kind="ExternalInput")
    wa_d = nc.dram_tensor("wa", [1, 2, 3, 256], F32, kind="ExternalInput")
    wu_d = nc.dram_tensor("wu", [128, 2, 4], F32, kind="ExternalInput")
    cu_d = nc.dram_tensor("cu", [4, 384], F32, kind="ExternalInput")
    wrt_d = nc.dram_tensor("wrt", [128, 2, 256], F32, kind="ExternalInput")
    br_d = nc.dram_tensor("br", [128, 2], F32, kind="ExternalInput")
    wsc_d = nc.dram_tensor("wsc", [128, 4, 2, 128], F32, kind="ExternalInput")
    bsc_d = nc.dram_tensor("bsc", [128, 2], F32, kind="ExternalInput")
    lam_d = nc.dram_tensor("lam", [4, 1], F32, kind="ExternalInput")
    fused_o = nc.dram_tensor("fused", [128, 2, H, W], BF16,
                             kind="ExternalOutput")
    cpr_o = nc.dram_tensor("cpr", [128, 2, H, W], BF16,
                           kind="ExternalOutput")

    with tile.TileContext(nc) as tc:
        _build_tile(tc, nc, make_identity, locals(), alpha, ws, bs)
    nc.compile()
    return nc


def _build_tile(tc, nc, make_identity, T, alpha, ws, bs):
    frm_d, oth_d, w3t_d, b3_d = T["frm_d"], T["oth_d"], T["w3t_d"], T["b3_d"]
    wa_d, wu_d, cu_d, wrt_d = T["wa_d"], T["wu_d"], T["cu_d"], T["wrt_d"]
    br_d, wsc_d, bsc_d, lam_d = T["br_d"], T["wsc_d"], T["bsc_d"], T["lam_d"]
    fused_o, cpr_o = T["fused_o"], T["cpr_o"]

    import contextlib
    ctx = contextlib.ExitStack()
    consts = ctx.enter_context(tc.tile_pool(name="consts", bufs=1))
    ew = ctx.enter_context(tc.tile_pool(name="ew", bufs=3))
    outr = ctx.enter_context(tc.tile_pool(name="outr", bufs=3))
    ps_conv = ctx.enter_context(tc.tile_pool(name="ps_conv", bufs=2,
                                             space="PSUM"))
    ps_sm = ctx.enter_context(tc.tile_pool(name="ps_sm", bufs=2,
                                           space="PSUM"))
    ps_fin = ctx.enter_context(tc.tile_pool(name="ps_fin", bufs=4,
                                            space="PSUM"))

    # ---- persistent SBUF tiles ----
    xin = consts.tile([128, 4, 66, 64], BF16)     # row-padded cat(frm,oth)
    w3t = consts.tile([128, 4, 2, 9, 128], BF16)  # conv weights
    fm = consts.tile([128, 2, 8, 512], BF16)      # conv output Fm
    e_sb = consts.tile([128, 2, 8, 512], BF16)    # E' = k*oth - frm
    t2_sb = consts.tile([128, 2, 8, 512], BF16)   # (1-a)*oth
    b3_sb = consts.tile([128, 2], F32)
    wa_sb = consts.tile([1, 2, 3, 256], F32)
    wu_sb = consts.tile([128, 2, 4], F32)
    cu_sb = consts.tile([4, 384], F32)
    wrt_sb = consts.tile([128, 2, 256], F32)
    br_sb = consts.tile([128, 2], F32)
    wsc_sb = consts.tile([128, 4, 2, 128], F32)
    bsc_sb = consts.tile([128, 2], F32)
    lam_sb = consts.tile([4, 1], F32)
    # stats
    sums1 = consts.tile([128, 2, 8], F32)      # per-(ct,pt) channel sums
    pp_sum = consts.tile([128, 2, 64, 8], F32)  # per-pt sum over h, pt inner
    pp_max = consts.tile([128, 2, 64, 8], F32)
    pq_sum = consts.tile([128, 2, 2, 64], F32)  # [m2|m3][ct] pooled sums
    pq_max = consts.tile([128, 2, 2, 64], F32)
    stat1 = consts.tile([128, 4], F32)          # [sum ct0, sum ct1, max...]
    # U chain (pooled rows kept on partition 0 only)
    row1s = consts.tile([1, 256], F32)
    row1m = consts.tile([1, 256], F32)
    row23s = consts.tile([1, 128], F32)
    row23m = consts.tile([1, 128], F32)
    a_sb = consts.tile([128, 2, 384], F32)
    ub_sb = consts.tile([4, 384], F32)
    ue_sb = consts.tile([4, 384], F32)
    rec_sb = consts.tile([1, 384], F32)
    u_all = consts.tile([4, 384], F32)
    u1n = consts.tile([128, 2, 4], F32)
    gag = consts.tile([128, 4], F32)
    spec = consts.tile([128, 2], F32)
    spcA = consts.tile([128, 2], F32)
    mx2 = consts.tile([128, 128], F32)
    G = consts.tile([4, HW], BF16)    # spatial: U3[h]*U2[w]
    Gc = consts.tile([4, HW], BF16)   # cp recon: U2[h]*U3[w]
    Gs = consts.tile([4, HW], BF16)   # Gc * s  (spatial sigmoid folded)
    s4 = consts.tile([4, HW], BF16)   # sigmoid spatial map, 4 rows
    MT = consts.tile([4, 256], BF16)
    brsr = consts.tile([1, 256], BF16)
    spcAb = consts.tile([1, 256], BF16)

    ident = consts.tile([128, 128], F32)
    make_identity(nc, ident[:])
    ones128 = consts.tile([128, 1], F32)
    nc.gpsimd.memset(ones128[:], 1.0)
    ones41 = consts.tile([4, 1], F32)
    nc.gpsimd.memset(ones41[:], 1.0)
    ones14 = consts.tile([1, 4], F32)
    nc.gpsimd.memset(ones14[:], 1.0)
    ones44b = consts.tile([4, 4], BF16)
    nc.gpsimd.memset(ones44b[:], 1.0)
    ones1x128b = consts.tile([1, 128], BF16)
    nc.gpsimd.memset(ones1x128b[:], 1.0)

    # ---- zero the padded top/bottom rows (cols handled per-tap) ----
    nc.gpsimd.memset(xin[:, :, 0:1, :], 0.0)
    nc.gpsimd.memset(xin[:, :, 65:66, :], 0.0)

    # ---- DMA order: b3 first, then per-kt (weights ct0, image h0) pairs,
    # then ct1 weights, h1 image halves, then smalls ----
    srcs = [frm_d, frm_d, oth_d, oth_d]
    for kt in range(4):
        if kt == 0:
            # tiny sliver for the first tap so the PE starts ASAP
            nc.scalar.dma_start(w3t[:, 0, 0, 4:5, :], w3t_d[:, 0, 0, 4:5, :])
            nc.scalar.dma_start(w3t[:, 0, 0, 0:4, :], w3t_d[:, 0, 0, 0:4, :])
            nc.scalar.dma_start(w3t[:, 0, 0, 5:9, :], w3t_d[:, 0, 0, 5:9, :])
        else:
            nc.scalar.dma_start(w3t[:, kt, 0], w3t_d[:, kt, 0])
        nc.sync.dma_start(xin[:, kt, 1:33, :],
                          srcs[kt][:, kt % 2, 0:32, :])
        if kt == 0:
            nc.scalar.dma_start(b3_sb[:], b3_d[:])
    for kt in range(4):
        nc.scalar.dma_start(w3t[:, kt, 1], w3t_d[:, kt, 1])
    for kt in range(4):
        nc.sync.dma_start(xin[:, kt, 33:65, :],
                          srcs[kt][:, kt % 2, 32:64, :])
    for sb, dd in ((wa_sb, wa_d), (wu_sb, wu_d), (cu_sb, cu_d),
                   (wrt_sb, wrt_d), (br_sb, br_d), (wsc_sb, wsc_d),
                   (bsc_sb, bsc_d), (lam_sb, lam_d)):
        nc.scalar.dma_start(sb[:], dd[:])

    kk_e = float((1.0 - alpha) / alpha)

    # ---- conv3x3 + lrelu + streaming stats ----
    for pt in range(8):
        for ct in range(2):
            ps = ps_conv.tile([128, 8, 64], F32, tag="conv")
            idx = 0
            for kt in range(4):
                for (dy, dx) in TAPS:
                    # column edge handling: dx=0 drops out-col 0, dx=2
                    # drops out-col 63 (zero contribution at the border)
                    co0, co1 = (1, 64) if dx == 0 else (0, 63) \
                        if dx == 2 else (0, 64)
                    ci0 = dx - 1 + co0
                    nc.tensor.matmul(
                        ps[:, :, co0:co1],
                        w3t[:, kt, ct, dy * 3 + dx, :],
                        xin[:, kt, pt * 8 + dy: pt * 8 + dy + 8,
                            ci0: ci0 + co1 - co0],
                        start=(idx == 0), stop=(idx == 35))
                    idx += 1
            nc.scalar.activation(fm[:, ct, pt].rearrange(
                "p (h w) -> p h w", h=8), ps[:], AF.Lrelu,
                bias=b3_sb[:, ct:ct + 1], alpha=0.01,
                accum_out=sums1[:, ct, pt:pt + 1])
            blk = fm[:, ct, pt].rearrange("p (h w) -> p h w", h=8)
            blk_t = fm[:, ct, pt].rearrange("p (h w) -> p w h", h=8)
            # mode3 (per-h) stats: disjoint slices, written directly
            nc.vector.tensor_reduce(pq_sum[:, 1, ct, pt * 8:(pt + 1) * 8],
                                    blk, axis=AX.X, op=ALU.add)
            nc.vector.tensor_reduce(pq_max[:, 1, ct, pt * 8:(pt + 1) * 8],
                                    blk, axis=AX.X, op=ALU.max)
            # mode2 (per-w) partials, combined after the loop
            nc.vector.tensor_reduce(pp_sum[:, ct, :, pt], blk_t,
                                    axis=AX.X, op=ALU.add)
            nc.vector.tensor_reduce(pp_max[:, ct, :, pt], blk_t,
                                    axis=AX.X, op=ALU.max)
            # E'/t2 precompute (no conv dependency; fills idle slots).
            # pt7's are deferred past the stats combine (critical path).
            if pt < 7:
                nc.vector.scalar_tensor_tensor(
                    e_sb[:, ct, pt].rearrange("p (h w) -> p h w", h=8),
                    xin[:, 2 + ct, 1 + pt * 8: 9 + pt * 8, :], kk_e,
                    xin[:, ct, 1 + pt * 8: 9 + pt * 8, :],
                    op0=ALU.mult, op1=ALU.subtract)
                nc.vector.tensor_scalar(
                    t2_sb[:, ct, pt].rearrange("p (h w) -> p h w", h=8),
                    xin[:, 2 + ct, 1 + pt * 8: 9 + pt * 8, :],
                    float(1.0 - alpha), None, op0=ALU.mult)

    # ---- combine stats (global max = max over h of per-h maxes) ----
    nc.vector.tensor_reduce(stat1[:, 0:2], sums1[:], axis=AX.X, op=ALU.add)
    nc.vector.tensor_reduce(stat1[:, 2:4], pq_max[:, 1], axis=AX.X,
                            op=ALU.max)
    nc.vector.tensor_reduce(pq_sum[:, 0], pp_sum[:], axis=AX.X, op=ALU.add)
    nc.vector.tensor_reduce(pq_max[:, 0], pp_max[:], axis=AX.X, op=ALU.max)

    # mode1 rows via per-column transposes; avg-scales folded into wa
    rowdst = [(row1s, 0), (row1s, 128), (row1m, 0), (row1m, 128)]
    for j, (dst, off) in enumerate(rowdst):
        tpj = ps_sm.tile([1, 128], F32, tag="sm")
        nc.tensor.transpose(tpj[:], stat1[:, j:j + 1], ident[:])
        nc.scalar.copy(dst[0:1, off:off + 128], tpj[:])

    # mode2/3 sum rows: ones-matmul over channels, add ct halves
    srow = ps_sm.tile([1, 2, 2, 64], F32, tag="sm")
    nc.tensor.matmul(srow[:].rearrange("p a b c -> p (a b c)"), ones128[:],
                     pq_sum[:].rearrange("p a b c -> p (a b c)"),
                     start=True, stop=True)
    sr_sb = consts.tile([1, 2, 2, 64], F32)
    nc.scalar.copy(sr_sb[:], srow[:])
    nc.vector.tensor_tensor(row23s[0:1, :].rearrange("p (m w) -> p m w",
                                                     m=2),
                            sr_sb[:, :, 0, :], sr_sb[:, :, 1, :], op=ALU.add)
    # mode2/3 max rows: ct-combine, transpose, reduce, transpose back
    nc.vector.tensor_tensor(mx2[:].rearrange("p (m w) -> p m w", m=2),
                            pq_max[:, :, 0, :], pq_max[:, :, 1, :],
                            op=ALU.max)
    mxT = ps_sm.tile([128, 128], F32, tag="sm")
    nc.tensor.transpose(mxT[:], mx2[:], ident[:])
    mcol = ew.tile([128, 1], F32, tag="mcol")
    nc.vector.tensor_reduce(mcol[:], mxT[:], axis=AX.X, op=ALU.max)
    mrow = ps_sm.tile([1, 128], F32, tag="sm")
    nc.tensor.transpose(mrow[:], mcol[:], ident[:])
    nc.scalar.copy(row23m[:], mrow[:])

    # ---- a[o, n] = sum_s wa_s[o] * row_s[n]  (rank-1 outer products) ----
    for ct in range(2):
        ap_t = ps_sm.tile([128, 384], F32, tag="sm")
        cs = ct * 128
        nc.tensor.matmul(ap_t[:, 0:256], wa_sb[0:1, 0, 0, cs:cs + 128],
                         row1s[:], start=True, stop=False)
        nc.tensor.matmul(ap_t[:, 0:256], wa_sb[0:1, 1, 0, cs:cs + 128],
                         row1m[:], start=False, stop=True)
        nc.tensor.matmul(ap_t[:, 256:320], wa_sb[0:1, 0, 1, cs:cs + 128],
                         row23s[:, 0:64], start=True, stop=False)
        nc.tensor.matmul(ap_t[:, 256:320], wa_sb[0:1, 1, 1, cs:cs + 128],
                         row23m[:, 0:64], start=False, stop=True)
        nc.tensor.matmul(ap_t[:, 320:384], wa_sb[0:1, 0, 2, cs:cs + 128],
                         row23s[:, 64:128], start=True, stop=False)
        nc.tensor.matmul(ap_t[:, 320:384], wa_sb[0:1, 1, 2, cs:cs + 128],
                         row23m[:, 64:128], start=False, stop=True)
        nc.scalar.copy(a_sb[:, ct, :], ap_t[:])

    # ---- u = Wu @ a + (Wu@ba + bu)  [4, 384], then softmax over k ----
    u_ps = ps_sm.tile([4, 384], F32, tag="sm")
    nc.tensor.matmul(u_ps[:], wu_sb[:, 0, :], a_sb[:, 0, :], start=True,
                     stop=False)
    nc.tensor.matmul(u_ps[:], wu_sb[:, 1, :], a_sb[:, 1, :], start=False,
                     stop=True)
    nc.vector.tensor_tensor(ub_sb[:], u_ps[:], cu_sb[:], op=ALU.add)
    nc.scalar.activation(ue_sb[:], ub_sb[:], AF.Exp)
    ssum = ps_sm.tile([1, 384], F32, tag="sm")
    nc.tensor.matmul(ssum[:], ones41[:], ue_sb[:], start=True, stop=True)
    nc.scalar.copy(rec_sb[:], ssum[:])
    rb = ps_sm.tile([4, 384], F32, tag="sm")
    nc.tensor.matmul(rb[:], ones14[:], rec_sb[:], start=True, stop=True)
    rcp4 = consts.tile([4, 384], F32)
    nc.vector.reciprocal_approx_fast(rcp4[:], rb[:])
    nc.vector.tensor_tensor(u_all[:], ue_sb[:], rcp4[:], op=ALU.mult)

    # G[r, h, w] = U3[h, r] * U2[w, r] (spatial); Gc swaps h/w roles
    # (built in per-pt chunks inside the final loop to pipeline)

    # ---- MT = (Wr @ U1 diag(lam))^T  [4, 256] bf16 ----
    for kk2 in range(2):
        u1t_ps = ps_sm.tile([128, 4], F32, tag="sm")
        nc.tensor.transpose(u1t_ps[:], u_all[0:4, kk2 * 128:(kk2 + 1) * 128],
                            ident[0:4, 0:4])
        nc.scalar.copy(u1n[:, kk2, :], u1t_ps[:])
    mt_ps = ps_sm.tile([4, 256], F32, tag="sm")
    nc.tensor.matmul(mt_ps[:], u1n[:, 0, :], wrt_sb[:, 0, :], start=True,
                     stop=False)
    nc.tensor.matmul(mt_ps[:], u1n[:, 1, :], wrt_sb[:, 1, :], start=False,
                     stop=True)
    nc.vector.tensor_scalar(MT[:], mt_ps[:], lam_sb[:], None, op0=ALU.mult)

    # ---- spectral attention ----
    f_ps = ps_sm.tile([128, 2, 128], F32, tag="sm")
    for ct in range(2):
        nc.tensor.matmul(f_ps[:, ct, :],
                         u_all[0:4, ct * 128:(ct + 1) * 128],
                         u_all[:, 256:384], start=True, stop=True)
        nc.vector.tensor_reduce(gag[:, ct:ct + 1], f_ps[:, ct, :],
                                axis=AX.X, op=ALU.add)
        nc.vector.tensor_reduce(gag[:, 2 + ct:3 + ct], f_ps[:, ct, :],
                                axis=AX.X, op=ALU.max)
    spv = ps_sm.tile([128, 2], F32, tag="sm")
    for mm in range(2):
        for kk2 in range(4):
            nc.tensor.matmul(spv[:, mm:mm + 1], wsc_sb[:, kk2, mm, :],
                             gag[:, kk2:kk2 + 1], start=(kk2 == 0),
                             stop=(kk2 == 3))
    for mm in range(2):
        stmp = ew.tile([128, 1], F32, tag="stmp")
        nc.scalar.activation(stmp[:], spv[:, mm:mm + 1], AF.Sigmoid,
                             bias=bsc_sb[:, mm:mm + 1])
        nc.scalar.activation(spec[:, mm:mm + 1], stmp[:], AF.Sigmoid)
    nc.vector.tensor_scalar(spcA[:], spec[:], float(-alpha), None,
                            op0=ALU.mult)
    # brs row: (br * spectral) as a [1, 256] bf16 row for the rank-1 term
    brs = ew.tile([128, 2], F32, tag="brs")
    nc.vector.tensor_tensor(brs[:], br_sb[:], spec[:], op=ALU.mult)
    for ct in range(2):
        brt = ps_sm.tile([1, 128], F32, tag="sm")
        nc.tensor.transpose(brt[:], brs[:, ct:ct + 1], ident[:])
        nc.vector.tensor_copy(brsr[0:1, ct * 128:(ct + 1) * 128], brt[:])
        sat = ps_sm.tile([1, 128], F32, tag="sm")
        nc.tensor.transpose(sat[:], spcA[:, ct:ct + 1], ident[:])
        nc.vector.tensor_copy(spcAb[0:1, ct * 128:(ct + 1) * 128], sat[:])
    for ct in range(2):  # deferred pt7 E'/t2
        nc.vector.scalar_tensor_tensor(
            e_sb[:, ct, 7].rearrange("p (h w) -> p h w", h=8),
            xin[:, 2 + ct, 57:65, :], kk_e, xin[:, ct, 57:65, :],
            op0=ALU.mult, op1=ALU.subtract)
        nc.vector.tensor_scalar(
            t2_sb[:, ct, 7].rearrange("p (h w) -> p h w", h=8),
            xin[:, 2 + ct, 57:65, :], float(1.0 - alpha), None,
            op0=ALU.mult)

    # ---- final stage ----
    # fused  = t2 + spcA*(E' . s);  cp = spec*((MT@Gc).s + br.s) + Fm
    # loop 1: spatial map chunks + sigmoid (no act-table thrash)
    for pt in range(8):
        sl = slice(pt * 512, (pt + 1) * 512)
        hs = slice(pt * 8, pt * 8 + 8)
        nc.vector.tensor_tensor(
            G[:, sl].rearrange("p (h w) -> p h w", h=8),
            u_all[:, 320:384][:, hs][:, :, None].broadcast_to([4, 8, 64]),
            u_all[:, 256:320][:, None, :].broadcast_to([4, 8, 64]),
            op=ALU.mult)
        srow_ps = ps_fin.tile([4, 512], F32, tag="sb2")
        nc.tensor.matmul(srow_ps[:], ones44b[:], G[:, sl], start=True,
                         stop=True)
        nc.scalar.activation(s4[:, sl], srow_ps[:], AF.Sigmoid,
                             scale=float(ws), bias=float(bs))
        nc.vector.tensor_tensor(
            Gc[:, sl].rearrange("p (h w) -> p h w", h=8),
            u_all[:, 256:320][:, hs][:, :, None].broadcast_to([4, 8, 64]),
            u_all[:, 320:384][:, None, :].broadcast_to([4, 8, 64]),
            op=ALU.mult)
    # loop 2: fused/recon outputs
    for pt in range(8):
        sl = slice(pt * 512, (pt + 1) * 512)
        nc.vector.tensor_tensor(Gs[:, sl], Gc[:, sl], s4[:, sl],
                                op=ALU.mult)
        wts = []
        for ct in range(2):
            wt_ps = ps_fin.tile([128, 512], F32, tag="sb2")
            nc.tensor.matmul(wt_ps[:],
                             spcAb[0:1, ct * 128:(ct + 1) * 128],
                             s4[0:1, sl], start=True, stop=True)
            wt_sb = ew.tile([128, 512], BF16, tag="wtsb")
            nc.scalar.copy(wt_sb[:], wt_ps[:])
            wts.append(wt_sb)
        rcs = []
        for ct in range(2):
            rc_ps = ps_conv.tile([128, 512], F32, tag="conv")
            nc.tensor.matmul(rc_ps[:], MT[:, ct * 128:(ct + 1) * 128],
                             Gs[:, sl], start=True, stop=False)
            nc.tensor.matmul(rc_ps[:], brsr[0:1, ct * 128:(ct + 1) * 128],
                             s4[0:1, sl], start=False, stop=True)
            rcs.append(rc_ps)
        for ct in range(2):
            fa = ew.tile([128, 512], BF16, tag="fa")
            nc.vector.tensor_tensor(fa[:], e_sb[:, ct, pt, :],
                                    wts[ct][:], op=ALU.mult)
            fu = outr.tile([128, 8, 64], BF16, tag="fu")
            nc.vector.tensor_tensor(
                fu[:], fa[:].rearrange("p (h w) -> p h w", h=8),
                t2_sb[:, ct, pt].rearrange("p (h w) -> p h w", h=8),
                op=ALU.add)
            nc.sync.dma_start(fused_o[:, ct, pt * 8:(pt + 1) * 8, :], fu[:])
        for ct in range(2):
            cp = outr.tile([128, 8, 64], BF16, tag="cp")
            nc.vector.scalar_tensor_tensor(
                cp[:], rcs[ct][:].rearrange("p (h w) -> p h w", h=8),
                spec[:, ct:ct + 1],
                fm[:, ct, pt].rearrange("p (h w) -> p h w", h=8),
                op0=ALU.mult, op1=ALU.add)
            nc.sync.dma_start(cpr_o[:, ct, pt * 8:(pt + 1) * 8, :], cp[:])
    ctx.close()


def _prep_weights(W3, b3, Wa1, ba1, Wa2, ba2, Wa3, ba3, Wu, bu, Wr, br,
                  Wsa, bsa, Wsm, bsm):
    f = np.float32
    # w3t[p, kt, ct, t, co] = W3[ct*128+co, kt*128+p, dy, dx]
    w3t = np.ascontiguousarray(
        W3.reshape(2, 128, 4, 128, 9).transpose(3, 2, 0, 4, 1)).astype(BF)
    b3h = np.ascontiguousarray(b3.reshape(2, 128).T).astype(f)
    # wa[0, s, m, o] = Wa_m[o, s]; avg column scaled by 1/pool_n
    was = []
    for m, wv in enumerate((Wa1, Wa2, Wa3)):
        wv = np.array(wv, f).copy()
        wv[:, 0] /= (HW if m == 0 else C * H)
        was.append(wv)
    wa = np.ascontiguousarray(
        np.stack(was, axis=0).transpose(2, 0, 1)[None]).astype(f)
    # wu[p, ct, k] = Wu[k, ct*128+p]
    wu = np.ascontiguousarray(
        Wu.reshape(K, 2, 128).transpose(2, 1, 0)).astype(f)
    # cu[k, n] = (Wu @ ba_m + bu)[k] for n in mode-m block
    cus = [Wu @ bam + bu for bam in (ba1, ba2, ba3)]
    cu = np.concatenate([np.tile(cus[0][:, None], (1, 256)),
                         np.tile(cus[1][:, None], (1, 64)),
                         np.tile(cus[2][:, None], (1, 64))], axis=1)
    cu = np.ascontiguousarray(cu).astype(f)
    # wrt[p, kk, m] = Wr[m, kk*128+p]
    wrt = np.ascontiguousarray(
        Wr.reshape(256, 2, 128).transpose(2, 1, 0)).astype(f)
    brh = np.ascontiguousarray(br.reshape(2, 128).T).astype(f)
    # wsc[p, kk, mm, m]: kk<2 -> Wsa/(W+H) (mean folded), kk>=2 -> Wsm
    wsa_r = (Wsa / 128.0).reshape(2, 128, 2, 128).transpose(3, 2, 0, 1)
    wsm_r = Wsm.reshape(2, 128, 2, 128).transpose(3, 2, 0, 1)
    wsc = np.ascontiguousarray(
        np.concatenate([wsa_r, wsm_r], axis=1)).astype(f)
    bsc = np.ascontiguousarray((bsa + bsm).reshape(2, 128).T).astype(f)
    return dict(w3t=w3t, b3=b3h, wa=wa, wu=wu, cu=cu, wrt=wrt, br=brh,
                wsc=wsc, bsc=bsc)


_CACHE = {}


def kernel(frm_feat, other_feat, W3, b3, Wa1, ba1, Wa2, ba2, Wa3, ba3,
           Wu, bu, Wr, br, ws, bs, Wsa, bsa, Wsm, bsm, alpha, lam,
           _trace=False, _tmpdir=None):
    key = (float(alpha), float(ws), float(bs))
    if key not in _CACHE:
        _CACHE[key] = build_program(float(alpha), float(ws), float(bs))
    nc = _CACHE[key]

    wd = _prep_weights(np.asarray(W3, np.float32), np.asarray(b3),
                       np.asarray(Wa1), np.asarray(ba1), np.asarray(Wa2),
                       np.asarray(ba2), np.asarray(Wa3), np.asarray(ba3),
                       np.asarray(Wu), np.asarray(bu),
                       np.asarray(Wr, np.float32), np.asarray(br),
                       np.asarray(Wsa, np.float32), np.asarray(bsa),
                       np.asarray(Wsm, np.float32), np.asarray(bsm))
    wd["lam"] = np.asarray(lam, np.float32).reshape(4, 1)

    in_maps = []
    for b_i in range(NCORES):
        m = dict(wd)
        m["frm"] = frm_bat(frm_feat, b_i)
        m["oth"] = frm_bat(other_feat, b_i)
        in_maps.append(m)

    res = bass_utils.run_bass_kernel_spmd(
        nc, in_maps, core_ids=list(range(NCORES)), trace=_trace,
        tmpdir=_tmpdir)
    fused = np.stack([_unshard(res.results[i]["fused"])
                      for i in range(NCORES)])
    cpr = np.stack([_unshard(res.results[i]["cpr"])
                    for i in range(NCORES)])
    kernel._last_exec_time_ns = res.exec_time_ns
    kernel._last_results = res
    return fused, cpr


def frm_bat(x, b_i):
    """[B, 256, H, W] f32 -> [128, 2, H, W] bf16 for batch b_i."""
    return np.ascontiguousarray(
        np.asarray(x[b_i], np.float32).reshape(2, 128, H, W)
        .transpose(1, 0, 2, 3)).astype(BF)


def _unshard(a):
    """[128, 2, H, W] bf16 -> [256, H, W] f32."""
    return np.ascontiguousarray(
        np.asarray(a).transpose(1, 0, 2, 3)).reshape(256, H, W)\
        .astype(np.float32)
